# revision 23
# baseline (speedup 1.0000x reference)
"""MoE routing kernel for Trainium2 (8 NeuronCores, SPMD expert-parallel).

Contract: kernel(**full_inputs) -> full output [B, S, H] float32.

Strategy
--------
- Host: compute the (tiny) gate + group-topk routing in numpy (bit-identical
  selection to the jax reference), build the per-(token,expert) combine
  weights, and dispatch: gather each expert's tokens into a padded,
  transposed buffer.  This is the "all-to-all by topk_idx" of the
  sharding hint, done at input-sharding time.
- Device (SPMD over 8 cores): core c holds experts (2c, 2c+1) and a 1/8
  TOKEN-slice of the shared expert.  The routed phase runs first: the
  SwiGLU MLP for its two experts over their gathered tokens (unweighted).
  The shared phase runs last over the core's 512-token slice with the FULL
  shared weights.
- FP8 hybrid: the first N8 (of 11) I-blocks of every routed expert's
  gate/up projections run as fp8-e4m3 DoubleRow matmuls (K=256 per
  instruction -> 2x PE rate); the rest and the whole down/shared path
  stay bf16.  Power-of-2 pre-scales (x*32, W*4096) keep e4m3 out of
  subnormals; the descale folds exactly into the silu activation's scale
  and into host-prescaled down-proj columns, so no extra device work.
  Measured end-to-end rel err ~1.86e-2 at N8=5 (gate 2e-2).
- Host: scale per-expert outputs by routing weights, scatter-add over
  token indices, place each core's shared token-slice, transpose back.

All bf16 matmuls accumulate fp32 in PSUM.  Weight panels are pre-tiled on
the host into the exact SBUF tile layout so each streams from HBM exactly
once as a contiguous per-partition DMA.  The fp8 blocks run FIRST (np-
outer), so the warmup window only needs the small fp8 tiles -> earlier
first matmul and no cold-window DMA starvation.
"""

import math

import numpy as np
import ml_dtypes

H = 2048          # hidden size
I = 1408          # intermediate per routed expert
E = 16            # routed experts
G = 4             # groups
TOPK_GROUP = 2
TOP_K = 6
N_SHARED = 2
SCALE_FACTOR = 2.5
SI = I * N_SHARED  # 2816 shared intermediate
N_CORES = 8
EXP_PER_CORE = E // N_CORES  # 2
P = 128
BF16 = ml_dtypes.bfloat16
FP8 = ml_dtypes.float8_e4m3fn

N8 = 5            # fp8 gate/up I-blocks per expert (of MI=11)
SX = 32.0         # x pre-scale for e4m3
SW = 4096.0       # weight pre-scale for e4m3
SINV = 1.0 / (SX * SW)

_COMPILED = {}  # (T, caps) -> nc
_LAST = {}      # debug/profiling handle for test.py


def _gate_host(hs, gate_weight, bias):
    """numpy replica of reference._gate (verified bit-identical selection)."""
    T = hs.shape[0]
    logits = hs @ gate_weight.T                       # [T, E] fp32
    scores = 1.0 / (1.0 + np.exp(-logits))
    sfc = scores + bias[None, :]
    gs = sfc.reshape(T, G, E // G)
    gsort = np.sort(gs, axis=-1)
    group_scores = gsort[..., -1] + gsort[..., -2]
    group_idx = np.argsort(-group_scores, axis=-1, kind="stable")[:, :TOPK_GROUP]
    gmask = np.zeros((T, G), bool)
    gmask[np.arange(T)[:, None], group_idx] = True
    smask = np.repeat(gmask, E // G, axis=1)
    tmp = np.where(smask, sfc, 0.0)
    topk_idx = np.argsort(-tmp, axis=-1, kind="stable")[:, :TOP_K]
    topk_w = np.take_along_axis(scores, topk_idx, axis=1)
    topk_w = topk_w / (topk_w.sum(-1, keepdims=True) + 1e-20) * SCALE_FACTOR
    return topk_idx.astype(np.int32), topk_w.astype(np.float32)


def _pw(w):
    return ((w + 15) // 16) * 16


def _build(T, caps):
    """Build + compile the SPMD Bass program.

    T    : total tokens; each core's shared slice is TS = T/4 of them
    caps : per expert slot, (C_cap, w): gathered-token capacity and matmul
           free-dim slice width; C_cap = NP_R * 2 * w
    """
    import concourse.mybir as mybir
    import concourse.tile as tile
    from concourse import bacc

    bf = mybir.dt.bfloat16
    f8 = mybir.dt.float8e4
    f32 = mybir.dt.float32
    AF = mybir.ActivationFunctionType
    DR = mybir.MatmulPerfMode.DoubleRow

    KH = H // P        # 16 contraction chunks over H
    KC = KH // 2       # 8 fp8 DoubleRow chunks (K=256 each)
    MI = I // P        # 11 I chunks
    MH = H // P        # 16 output H chunks
    MSI = SI // (2 * P)   # 11 shared-intermediate chunks (half columns)
    TS = T // 4           # 1024 token slice for the shared expert (D=2)
    assert TS == 1024
    for (C_cap, w) in caps:
        assert C_cap % (2 * w) == 0 and w <= 512
    C_tot = sum(C_cap for C_cap, _ in caps)
    slot_base = [sum(C for C, _ in caps[:s]) for s in range(len(caps))]
    w_max = max(w for _, w in caps)
    pws = [_pw(w) for _, w in caps]

    nc = bacc.Bacc("TRN2", target_bir_lowering=False, debug=False,
                   num_devices=N_CORES)
    # x panels are host-packed to [tile, p, kk*cols+c] so each tile loads as
    # one DMA with fat (multi-KB) contiguous per-partition rows
    xs = nc.dram_tensor("xs", [KH // 4, P, 4 * TS], bf, kind="ExternalInput")
    xgs = [
        nc.dram_tensor(f"xg{s}", [(C // (2 * w)) * (KH // 4), P, 4 * 2 * w],
                       bf, kind="ExternalInput")
        for s, (C, w) in enumerate(caps)
    ]
    # fp8 DoubleRow x panels: per np block, 4 tiles of [P, kk=2, i=2, 2*PW]
    # (chunk c=2q+kk contracts h = c*256 + i*128 + p; col = j*PW + n)
    xg8s = [
        nc.dram_tensor(f"xg8{s}", [(C // (2 * w)) * 4, P, 2 * 2 * 2 * pw],
                       f8, kind="ExternalInput")
        for s, ((C, w), pw) in enumerate(zip(caps, pws))
    ]
    # weight panels are pre-tiled on the host to the exact SBUF tile layout
    # bf16 gate/up panels exist only for the MI-N8 bf16 blocks
    wg = nc.dram_tensor("wg", [EXP_PER_CORE * (MI - N8), P, KH * P], bf,
                        kind="ExternalInput")
    wu = nc.dram_tensor("wu", [EXP_PER_CORE * (MI - N8), P, KH * P], bf,
                        kind="ExternalInput")
    # fp8 DoubleRow gate/up panels for the first N8 blocks: [P, c, i, m]
    wg8 = nc.dram_tensor("wg8", [EXP_PER_CORE * N8, P, KC * 2 * P], f8,
                         kind="ExternalInput")
    wu8 = nc.dram_tensor("wu8", [EXP_PER_CORE * N8, P, KC * 2 * P], f8,
                         kind="ExternalInput")
    wd = nc.dram_tensor("wd", [EXP_PER_CORE * MH, P, MI * P], bf,
                        kind="ExternalInput")
    sg = nc.dram_tensor("sg", [MSI, P, KH * P], bf, kind="ExternalInput")
    su = nc.dram_tensor("su", [MSI, P, KH * P], bf, kind="ExternalInput")
    sd = nc.dram_tensor("sd", [MH, P, MSI * P], bf, kind="ExternalInput")
    ye = nc.dram_tensor("ye", [H, C_tot], bf, kind="ExternalOutput")
    ys = nc.dram_tensor("ys", [H, TS], bf, kind="ExternalOutput")

    with tile.TileContext(nc) as tc:
        with (
            # xgp serves both the routed bf16 x quads AND (phase-disjoint,
            # via ring reuse) the shared-expert x tiles — both [128,4,1024]
            tc.tile_pool(name="xgp", bufs=8) as xgp,
            tc.tile_pool(name="x8p", bufs=9) as x8p,   # fp8 DR x tiles
            tc.tile_pool(name="wp", bufs=7) as wp,     # [128,16,128] bf16 w
            tc.tile_pool(name="w8p", bufs=10) as w8p,  # [128,8,2,128] fp8 w
            tc.tile_pool(name="wdp", bufs=2) as wdp,   # [128,11,128] down cols
            tc.tile_pool(name="sdp", bufs=3) as sdp,   # [128,22,128] shared down
            tc.tile_pool(name="itp", bufs=44) as itp,  # [128,512] bf16 inter
            tc.tile_pool(name="tmp", bufs=2) as tmp,   # silu temp
            tc.tile_pool(name="otp", bufs=3) as otp,   # [128,1024] bf16 out
            tc.tile_pool(name="pg", bufs=3, space="PSUM") as pgp,
            tc.tile_pool(name="pu", bufs=3, space="PSUM") as pup,
            tc.tile_pool(name="py", bufs=2, space="PSUM") as pyp,
        ):
            # PE clock warm-up: the HAM gate keeps the PE at 1.2 GHz until
            # ~3.4us of sustained activity.  The first real matmul is data-
            # bound at ~10.6us; 14 dummy matmuls on a memset tile fill the
            # 6.7-9.7us DMA window so the 2.4 GHz flip lands right as the
            # real stream starts.  (More dummies would outrun the DMA ramp
            # and idle-rethrottle — measured, don't extend.)
            warm = tmp.tile([P, 512], bf, name="warm", tag="tmp")
            nc.gpsimd.memset(warm[:], 0.0)
            wps = pyp.tile([P, 512], f32, name="warmps", tag="py")
            for _ in range(14):
                nc.tensor.matmul(wps[:], warm[:, :128], warm[:],
                                 start=True, stop=True)

            # Queue discipline: gpsimd carries ONLY data-dependent writes
            # (plus the late sd loads behind them), so no load trigger ever
            # queues behind a write's semaphore wait.  Loads ride
            # scalar+sync.
            def load_w8(s, wait=None, split_first=False, ms=None):
                """fp8 gate/up weight tiles for the N8 fp8 blocks."""
                tiles = {}
                with tc.tile_wait_until(wait, enable=wait is not None):
                    for m in (range(N8) if ms is None else ms):
                        wgt = w8p.tile([P, KC, 2, P], f8, name=f"w8g{s}_{m}",
                                       tag="w8")
                        src = wg8[s * N8 + m]
                        if split_first and m == 0:
                            nc.sync.dma_start(
                                wgt[:, :KC // 2],
                                src[:, :KC // 2 * 2 * P]
                                .rearrange("p (c i m) -> p c i m", i=2, m=P))
                            nc.sync.dma_start(
                                wgt[:, KC // 2:],
                                src[:, KC // 2 * 2 * P:]
                                .rearrange("p (c i m) -> p c i m", i=2, m=P))
                        else:
                            eng = (nc.sync, nc.scalar)[m % 2]
                            eng.dma_start(
                                wgt[:],
                                src.rearrange("p (c i m) -> p c i m",
                                              i=2, m=P))
                        wut = w8p.tile([P, KC, 2, P], f8, name=f"w8u{s}_{m}",
                                       tag="w8")
                        eng = (nc.scalar, nc.sync)[m % 2]
                        eng.dma_start(
                            wut[:],
                            wu8[s * N8 + m]
                            .rearrange("p (c i m) -> p c i m", i=2, m=P))
                        tiles[m] = (wgt, wut)
                return tiles

            def load_x8(s, wait=None, nps=None):
                """fp8 DR x tiles: 4 per np block, each 2 chunks."""
                C_cap, w = caps[s]
                pw = pws[s]
                NP_R = C_cap // (2 * w)
                t8 = {}
                with tc.tile_wait_until(wait, enable=wait is not None):
                    for np_ in (range(NP_R) if nps is None else nps):
                        for q in range(4):
                            if s == 0 and np_ == 0 and q == 0 and wait is None:
                                # warmup: kk-split halves so chain k=0 can
                                # fire after only 0.21MB
                                ta = x8p.tile([P, 1, 2, 2 * pw], f8,
                                              name=f"x8{s}_0_0a", tag="x8")
                                nc.scalar.dma_start(
                                    ta[:],
                                    xg8s[s][0][:, :2 * 2 * pw]
                                    .rearrange("p (kk i c) -> p kk i c",
                                               i=2, c=2 * pw))
                                tb = x8p.tile([P, 1, 2, 2 * pw], f8,
                                              name=f"x8{s}_0_0b", tag="x8")
                                nc.sync.dma_start(
                                    tb[:],
                                    xg8s[s][0][:, 2 * 2 * pw:]
                                    .rearrange("p (kk i c) -> p kk i c",
                                               i=2, c=2 * pw))
                                t8[(np_, q)] = ("split", ta, tb)
                                continue
                            t = x8p.tile([P, 2, 2, 2 * pw], f8,
                                         name=f"x8{s}_{np_}_{q}", tag="x8")
                            eng = (nc.scalar, nc.sync)[q % 2]
                            eng.dma_start(
                                t[:],
                                xg8s[s][np_ * 4 + q]
                                .rearrange("p (kk i c) -> p kk i c",
                                           i=2, c=2 * pw))
                            t8[(np_, q)] = t
                return t8

            def load_x(s, wait=None):
                """bf16 x quads (used by the MI-N8 bf16 blocks)."""
                C_cap, w = caps[s]
                NP_R = C_cap // (2 * w)
                xgt = {}
                with tc.tile_wait_until(wait, enable=wait is not None):
                    for np_ in range(NP_R):
                        for kq in range(KH // 4):
                            t = xgp.tile([P, 4, 1024], bf,
                                         name=f"xg{s}_{np_}_{kq}", tag="x")
                            eng = (nc.scalar, nc.sync)[kq % 2]
                            eng.dma_start(
                                t[:, :, :2 * w],
                                xgs[s][np_ * (KH // 4) + kq]
                                .rearrange("p (kk c) -> p kk c", c=2 * w))
                            xgt[(np_, kq)] = t
                return xgt

            def gate_up(s, xgt, x8t, w8tiles, wwait=None):
                C_cap, w = caps[s]
                pw = pws[s]
                NP_R = C_cap // (2 * w)
                inter = {}
                wtiles = {}

                def wts(m):
                    # bf16 weights for blocks m >= N8, loaded on first use
                    if m not in wtiles:
                        mi = s * (MI - N8) + (m - N8)
                        with tc.tile_wait_until(wwait,
                                                enable=wwait is not None):
                            wgt = wp.tile([P, KH, P], bf, name=f"wgt{s}_{m}",
                                          tag="wp")
                            nc.sync.dma_start(
                                wgt[:],
                                wg[mi].rearrange("p (ko c) -> p ko c", c=P))
                            wut = wp.tile([P, KH, P], bf, name=f"wut{s}_{m}",
                                          tag="wp")
                            nc.sync.dma_start(
                                wut[:],
                                wu[mi].rearrange("p (ko c) -> p ko c", c=P))
                        wtiles[m] = (wgt, wut)
                    return wtiles[m]

                def x8op(np_, k, j):
                    t = x8t[(np_, k // 2)]
                    if isinstance(t, tuple):
                        return t[1 + k % 2][:, 0, :, j * pw:j * pw + w]
                    return t[:, k % 2, :, j * pw:j * pw + w]

                # --- fp8 DoubleRow blocks, np-outer so warmup reuses np0 x ---
                for np_ in range(NP_R):
                    for m in range(N8):
                        wgt, wut = w8tiles[m]
                        for j in range(2):
                            psg = pgp.tile([P, 512], f32,
                                           name=f"psg{s}_{m}_{np_}{j}",
                                           tag="pg")
                            for k in range(KC):
                                nc.tensor.matmul(
                                    psg[:, :w], wgt[:, k, :, :],
                                    x8op(np_, k, j),
                                    start=(k == 0), stop=(k == KC - 1),
                                    perf_mode=DR)
                            st = tmp.tile([P, 512], bf,
                                          name=f"st{s}_{m}_{np_}{j}",
                                          tag="tmp")
                            nc.scalar.activation(st[:, :w], psg[:, :w],
                                                 AF.Silu, scale=SINV)
                            psu = pup.tile([P, 512], f32,
                                           name=f"psu{s}_{m}_{np_}{j}",
                                           tag="pu")
                            for k in range(KC):
                                nc.tensor.matmul(
                                    psu[:, :w], wut[:, k, :, :],
                                    x8op(np_, k, j),
                                    start=(k == 0), stop=(k == KC - 1),
                                    perf_mode=DR)
                            it = itp.tile([P, 512], bf,
                                          name=f"it{s}_{m}_{np_}{j}",
                                          tag="it")
                            # it = silu(g) * (u * SX*SW); the down-proj
                            # columns for this block are host-divided by
                            # SX*SW, so the product is exact
                            nc.vector.tensor_mul(it[:, :w], st[:, :w],
                                                 psu[:, :w])
                            inter[(m, np_, j)] = it

                # --- bf16 blocks (m-outer so weight tiles rotate through
                # the wp ring; all x quads are resident by this point) ---
                for m in range(N8, MI):
                    wgt, wut = wts(m)
                    for np_ in range(NP_R):
                        for j in range(2):
                            psg = pgp.tile([P, 512], f32,
                                           name=f"psg{s}_{m}_{np_}{j}",
                                           tag="pg")
                            for k in range(KH):
                                nc.tensor.matmul(
                                    psg[:, :w], wgt[:, k, :],
                                    xgt[(np_, k // 4)][:, k % 4,
                                                       j * w:(j + 1) * w],
                                    start=(k == 0), stop=(k == KH - 1))
                            st = tmp.tile([P, 512], bf,
                                          name=f"st{s}_{m}_{np_}{j}",
                                          tag="tmp")
                            nc.scalar.activation(st[:, :w], psg[:, :w],
                                                 AF.Silu)
                            psu = pup.tile([P, 512], f32,
                                           name=f"psu{s}_{m}_{np_}{j}",
                                           tag="pu")
                            for k in range(KH):
                                nc.tensor.matmul(
                                    psu[:, :w], wut[:, k, :],
                                    xgt[(np_, k // 4)][:, k % 4,
                                                       j * w:(j + 1) * w],
                                    start=(k == 0), stop=(k == KH - 1))
                            it = itp.tile([P, 512], bf,
                                          name=f"it{s}_{m}_{np_}{j}",
                                          tag="it")
                            nc.vector.tensor_mul(it[:, :w], st[:, :w],
                                                 psu[:, :w])
                            inter[(m, np_, j)] = it
                return inter

            def down(s, inter, wwait=None):
                C_cap, w = caps[s]
                NP_R = C_cap // (2 * w)
                for M in range(MH):
                    wdt = wdp.tile([P, MI, P], bf, name=f"wdt{s}_{M}", tag="wdt")
                    with tc.tile_wait_until(wwait, enable=wwait is not None):
                        nc.sync.dma_start(
                            wdt[:],
                            wd[s * MH + M].rearrange("p (ko c) -> p ko c", c=P))
                    for np_ in range(NP_R):
                        b0 = slot_base[s] + np_ * 2 * w
                        ot = otp.tile([P, 1024], bf,
                                      name=f"ot{s}_{M}_{np_}", tag="ot")
                        for j in range(2):
                            psy = pyp.tile([P, 512], f32,
                                           name=f"psy{s}_{M}_{np_}{j}",
                                           tag="py")
                            for K in range(MI):
                                nc.tensor.matmul(
                                    psy[:, :w], wdt[:, K, :],
                                    inter[(K, np_, j)][:, :w],
                                    start=(K == 0), stop=(K == MI - 1))
                            nc.vector.tensor_copy(
                                ot[:, j * w:(j + 1) * w], psy[:, :w])
                        nc.gpsimd.dma_start(
                            ye[M * P:(M + 1) * P, b0:b0 + 2 * w],
                            ot[:, :2 * w])

            # ---------------- routed experts ----------------
            # tile_wait_until floors (compile-time scheduler hints, in ms)
            # keep non-urgent loads out of the warmup window so slot 0's
            # fp8 stream owns the DMA bandwidth early on
            # warmup DMA order: m0's fp8 weights, np0's fp8 x, the rest of
            # the fp8 weights, np1's fp8 x — the first chains' data never
            # queues behind bytes they don't need yet
            w8_0 = load_w8(0, split_first=True, ms=[0])
            x8_0 = load_x8(0, nps=[0])
            w8_0.update(load_w8(0, ms=list(range(1, N8))))
            x8_0.update(load_x8(0, nps=[1]))
            xgt0 = load_x(0, wait=0.012)
            w8_1 = load_w8(1, wait=0.16)
            x8_1 = load_x8(1, wait=0.18)
            inter0 = gate_up(0, xgt0, x8_0, w8_0, wwait=0.015)
            xgt1 = load_x(1, wait=0.22)
            down(0, inter0, wwait=0.10)
            inter1 = gate_up(1, xgt1, x8_1, w8_1, wwait=0.26)
            down(1, inter1, wwait=0.40)

            # ------- shared expert (D=2: half columns x 1024 tokens) -------
            xst = []
            with tc.tile_wait_until(0.42):
                for kq in range(KH // 4):
                    t = xgp.tile([P, 4, TS], bf, name=f"xs{kq}", tag="x")
                    nc.scalar.dma_start(
                        t[:], xs[kq].rearrange("p (kk c) -> p kk c", c=TS))
                    xst.append(t)
            sint = {}
            for m in range(MSI):
                sgt = wp.tile([P, KH, P], bf, name=f"sgt{m}", tag="wp")
                nc.sync.dma_start(
                    sgt[:], sg[m].rearrange("p (ko c) -> p ko c", c=P))
                sut = wp.tile([P, KH, P], bf, name=f"sut{m}", tag="wp")
                nc.sync.dma_start(
                    sut[:], su[m].rearrange("p (ko c) -> p ko c", c=P))
                for j in range(2):
                    psg = pgp.tile([P, 512], f32, name=f"psgs{m}{j}", tag="pg")
                    for k in range(KH):
                        nc.tensor.matmul(
                            psg[:], sgt[:, k, :],
                            xst[k // 4][:, k % 4, j * 512:(j + 1) * 512],
                            start=(k == 0), stop=(k == KH - 1))
                    st = tmp.tile([P, 512], bf, name=f"sts{m}{j}", tag="tmp")
                    nc.scalar.activation(st[:], psg[:], AF.Silu)
                    psu = pup.tile([P, 512], f32, name=f"psus{m}{j}", tag="pu")
                    for k in range(KH):
                        nc.tensor.matmul(
                            psu[:], sut[:, k, :],
                            xst[k // 4][:, k % 4, j * 512:(j + 1) * 512],
                            start=(k == 0), stop=(k == KH - 1))
                    it = itp.tile([P, 512], bf, name=f"si{m}{j}", tag="it")
                    nc.vector.tensor_mul(it[:], st[:], psu[:])
                    sint[(m, j)] = it
            for M in range(MH):
                sdt = sdp.tile([P, MSI, P], bf, name=f"sdt{M}", tag="sdt")
                with tc.tile_wait_until(0.60):
                    nc.gpsimd.dma_start(
                        sdt[:], sd[M].rearrange("p (ko c) -> p ko c", c=P))
                ot = otp.tile([P, 1024], bf, name=f"ots{M}", tag="ot")
                # last M drains in smaller chunks so the final cast+DMA
                # tail after the last matmul is shorter
                chunks = ([(0, 512), (512, 512)] if M < MH - 1
                          else [(0, 512), (512, 256), (768, 256)])
                for ci, (c0, cw) in enumerate(chunks):
                    j = min(c0 // 512, 1)
                    psy = pyp.tile([P, 512], f32, name=f"psys{M}_{ci}",
                                   tag="py")
                    for K in range(MSI):
                        nc.tensor.matmul(
                            psy[:, :cw], sdt[:, K, :],
                            sint[(K, j)][:, c0 - j * 512:c0 - j * 512 + cw],
                            start=(K == 0), stop=(K == MSI - 1))
                    nc.vector.tensor_copy(ot[:, c0:c0 + cw], psy[:, :cw])
                    nc.scalar.dma_start(
                        ys[M * P:(M + 1) * P, c0:c0 + cw],
                        ot[:, c0:c0 + cw])

    nc.compile()
    return nc


def _get_compiled(T, caps):
    key = (T, tuple(caps))
    if key not in _COMPILED:
        _COMPILED[key] = _build(T, caps)
    return _COMPILED[key]


def _cap_for(maxc):
    maxc = max(int(maxc), 64)
    np_r = max(2, math.ceil(maxc / 2048))
    w = min(512, 2 * math.ceil(maxc / (np_r * 2 * 2)))
    C_cap = np_r * 2 * w
    assert C_cap >= maxc
    return C_cap, w


def kernel(hidden_states, gate_weight, e_score_correction_bias,
           gate_proj, up_proj, down_proj,
           shared_gate_w, shared_up_w, shared_down_w):
    from concourse.bass_utils import run_bass_kernel_spmd

    hs = np.asarray(hidden_states, dtype=np.float32)
    B, S, Hh = hs.shape
    assert Hh == H
    hsf = np.ascontiguousarray(hs.reshape(-1, H))
    T = hsf.shape[0]
    TS = T // 4        # shared-expert token slice (D=2 hybrid shard)
    gate_weight = np.asarray(gate_weight, np.float32)
    bias = np.asarray(e_score_correction_bias, np.float32)
    gate_proj = np.asarray(gate_proj, np.float32)
    up_proj = np.asarray(up_proj, np.float32)
    down_proj = np.asarray(down_proj, np.float32)
    shared_gate_w = np.asarray(shared_gate_w, np.float32)
    shared_up_w = np.asarray(shared_up_w, np.float32)
    shared_down_w = np.asarray(shared_down_w, np.float32)

    # ---- routing on host ----
    topk_idx, topk_w = _gate_host(hsf, gate_weight, bias)
    comb = np.zeros((T, E), np.float32)
    np.add.at(comb, (np.arange(T)[:, None], topk_idx), topk_w)
    sel = np.zeros((T, E), bool)
    sel[np.arange(T)[:, None], topk_idx] = True
    idx_e = [np.nonzero(sel[:, e])[0] for e in range(E)]
    counts = np.array([len(ix) for ix in idx_e])

    # assign experts to (core, slot): slot 0 gets the 8 largest, slot 1 the
    # 8 smallest, so each slot's capacity (uniform across cores under SPMD)
    # hugs its own max count
    order = np.argsort(-counts, kind="stable")
    assign = np.zeros((N_CORES, EXP_PER_CORE), np.int64)
    for c in range(N_CORES):
        assign[c, 0] = order[c]
        assign[c, 1] = order[2 * N_CORES - 1 - c]
    caps = [
        _cap_for(counts[assign[:, 0]].max()),
        _cap_for(counts[assign[:, 1]].max()),
    ]
    slot_base = [0, caps[0][0]]
    C_tot = caps[0][0] + caps[1][0]

    # ---- host-side dispatch (shard + transpose + bf16/fp8 cast) ----
    xsT = np.ascontiguousarray(hsf.T)                       # [H, T] fp32
    xsTb = xsT.astype(BF16)
    xsT8 = np.clip(xsT * SX, -240, 240).astype(FP8)

    MI, MH, MSI, KH = I // P, H // P, SI // (2 * P), H // P
    KC = KH // 2
    SIH = SI // 2      # 1408 shared-intermediate columns per group
    NF8 = N8 * P       # fp8 I-rows per expert

    def tile_gu(wmat, nm):  # [I', H] -> [nm, P, KH*P] : (m, p_h, ko_h*P + c_i)
        return np.ascontiguousarray(
            wmat.reshape(nm, P, KH, P).transpose(0, 3, 2, 1)
        ).reshape(nm, P, KH * P).astype(BF16)

    def tile_gu8(wmat):  # [NF8, H] fp32 -> [N8, P, KC*2*P] fp8 DR layout
        q = np.clip(wmat * SW, -240, 240).astype(FP8)
        # [m, r, c, i, p] -> [m, p, c, i, r]
        a = q.reshape(N8, P, KC, 2, P).transpose(0, 4, 2, 3, 1)
        return np.ascontiguousarray(a).reshape(N8, P, KC * 2 * P)

    def tile_dn(wmat, nk):  # [H, I'] -> [MH, P, nk*P] : (M, p_i, Ko_i*P + c_h)
        return np.ascontiguousarray(
            wmat.reshape(MH, P, nk, P).transpose(0, 3, 2, 1)
        ).reshape(MH, P, nk * P).astype(BF16)

    # shared weights: two column groups (cores 0-3 and 4-7); each core also
    # takes a 1024-token slice, so the shared output is a 2-way partial sum
    sg_g = [tile_gu(shared_gate_w[g * SIH:(g + 1) * SIH], MSI) for g in (0, 1)]
    su_g = [tile_gu(shared_up_w[g * SIH:(g + 1) * SIH], MSI) for g in (0, 1)]
    sd_g = [tile_dn(shared_down_w[:, g * SIH:(g + 1) * SIH], MSI)
            for g in (0, 1)]

    def pack_panels(xmat, NP, cols):
        # [H, NP*cols] -> [NP*4, P, 4*cols], tile np*4+kq holds h rows
        # (kq*4+kk)*128+p and cols [np*cols + c]
        a = xmat.reshape(4, 4, P, NP, cols)          # [kq, kk, p, np, c]
        return np.ascontiguousarray(
            a.transpose(3, 0, 2, 1, 4).reshape(NP * 4, P, 4 * cols)
        )

    def pack_panels8(x8, NP, w, pw):
        # [H, NP*2w] fp8 -> [NP*4, P, 2*2*2pw]: tile np*4+q holds chunks
        # c = 2q+kk (h = c*256 + i*128 + p), cols j*pw + n (n < w)
        a = x8.reshape(8, 2, P, NP, 2, w)            # [c, i, p, np, j, n]
        out = np.zeros((NP, 4, 2, P, 2, 2, pw), FP8)  # [np,q,kk,p,i,j,col]
        out[..., :w] = a.transpose(3, 0, 2, 1, 4, 5).reshape(
            NP, 4, 2, P, 2, 2, w)
        return np.ascontiguousarray(
            out.transpose(0, 1, 3, 2, 4, 5, 6).reshape(NP * 4, P, 2 * 2 * 2 * pw))

    in_maps = []
    for c in range(N_CORES):
        e0, e1 = assign[c]
        xg_pk, xg8_pk = [], []
        for sslot, e in enumerate((e0, e1)):
            C_cap, w = caps[sslot]
            pw = _pw(w)
            NP_R = C_cap // (2 * w)
            xg_c = np.zeros((H, C_cap), BF16)
            xg_c[:, :counts[e]] = xsTb[:, idx_e[e]]
            xg_pk.append(pack_panels(xg_c, NP_R, 2 * w))
            xg8_c = np.zeros((H, C_cap), FP8)
            xg8_c[:, :counts[e]] = xsT8[:, idx_e[e]]
            xg8_pk.append(pack_panels8(xg8_c, NP_R, w, pw))
        wg_c = np.concatenate([tile_gu(gate_proj[e][NF8:], MI - N8)
                               for e in (e0, e1)])
        wu_c = np.concatenate([tile_gu(up_proj[e][NF8:], MI - N8)
                               for e in (e0, e1)])
        wg8_c = np.concatenate([tile_gu8(gate_proj[e][:NF8])
                                for e in (e0, e1)])
        wu8_c = np.concatenate([tile_gu8(up_proj[e][:NF8])
                                for e in (e0, e1)])
        wd_list = []
        for e in (e0, e1):
            dpe = down_proj[e].copy()
            dpe[:, :NF8] *= SINV      # descale for fp8 blocks' inter
            wd_list.append(tile_dn(dpe, MI))
        wd_c = np.concatenate(wd_list)
        g, ts = c // 4, c % 4
        in_maps.append({
            "xs": pack_panels(
                np.ascontiguousarray(xsTb[:, ts * TS:(ts + 1) * TS]), 1, TS),
            "xg0": xg_pk[0], "xg1": xg_pk[1],
            "xg80": xg8_pk[0], "xg81": xg8_pk[1],
            "wg": wg_c, "wu": wu_c, "wg8": wg8_c, "wu8": wu8_c, "wd": wd_c,
            "sg": sg_g[g], "su": su_g[g], "sd": sd_g[g],
        })

    nc = _get_compiled(T, caps)

    def _run_and_combine():
        results = run_bass_kernel_spmd(nc, in_maps,
                                       core_ids=list(range(N_CORES)))
        outT = np.zeros((H, T), np.float32)
        for c in range(N_CORES):
            ts = c % 4
            outT[:, ts * TS:(ts + 1) * TS] += \
                results.results[c]["ys"].astype(np.float32)
        for c in range(N_CORES):
            ye = results.results[c]["ye"].astype(np.float32)
            for sslot in range(EXP_PER_CORE):
                e = assign[c, sslot]
                cnt = counts[e]
                if cnt == 0:
                    continue
                b0 = slot_base[sslot]
                we = comb[idx_e[e], e]
                outT[:, idx_e[e]] += ye[:, b0:b0 + cnt] * we[None, :]
        return results, outT

    def _spot_err(outT):
        # exact fp32 recompute of a few tokens; the hybrid device path is
        # within ~3e-2 of this, a corrupted pass is off by orders of
        # magnitude
        errs = []
        for t in (0, T // 3, T // 2, T - 1):
            x = hsf[t]
            acc = np.zeros(H, np.float32)
            for e in topk_idx[t]:
                g = gate_proj[e] @ x
                u = up_proj[e] @ x
                inter = g / (1.0 + np.exp(-g)) * u
                acc += comb[t, e] * (down_proj[e] @ inter)
            sg_ = shared_gate_w @ x
            su_ = shared_up_w @ x
            acc += shared_down_w @ (sg_ / (1.0 + np.exp(-sg_)) * su_)
            errs.append(np.linalg.norm(outT[:, t] - acc)
                        / (np.linalg.norm(acc) + 1e-20))
        return max(errs)

    results, outT = _run_and_combine()
    if _spot_err(outT) > 0.2:   # transient device fault: retry once
        results, outT = _run_and_combine()

    _LAST.clear()
    _LAST.update(nc=nc, in_maps=in_maps, results=results, caps=caps)

    return np.ascontiguousarray(outT.T).reshape(B, S, H).astype(np.float32)


# revision 24
# speedup vs baseline: 1.0024x; 1.0024x over previous
"""MoE routing kernel for Trainium2 (8 NeuronCores, SPMD expert-parallel).

Contract: kernel(**full_inputs) -> full output [B, S, H] float32.

Strategy
--------
- Host: compute the (tiny) gate + group-topk routing in numpy (bit-identical
  selection to the jax reference), build the per-(token,expert) combine
  weights, and dispatch: gather each expert's tokens into a padded,
  transposed buffer.  This is the "all-to-all by topk_idx" of the
  sharding hint, done at input-sharding time.
- Device (SPMD over 8 cores): core c holds experts (2c, 2c+1) and a 1/8
  TOKEN-slice of the shared expert.  The routed phase runs first: the
  SwiGLU MLP for its two experts over their gathered tokens (unweighted).
  The shared phase runs last over the core's 512-token slice with the FULL
  shared weights.
- FP8 hybrid: the first N8 (of 11) I-blocks of every routed expert's
  gate/up projections run as fp8-e4m3 DoubleRow matmuls (K=256 per
  instruction -> 2x PE rate); the rest and the whole down/shared path
  stay bf16.  Power-of-2 pre-scales (x*32, W*4096) keep e4m3 out of
  subnormals; the descale folds exactly into the silu activation's scale
  and into host-prescaled down-proj columns, so no extra device work.
  Measured end-to-end rel err ~1.86e-2 at N8=5 (gate 2e-2).
- Host: scale per-expert outputs by routing weights, scatter-add over
  token indices, place each core's shared token-slice, transpose back.

All bf16 matmuls accumulate fp32 in PSUM.  Weight panels are pre-tiled on
the host into the exact SBUF tile layout so each streams from HBM exactly
once as a contiguous per-partition DMA.  The fp8 blocks run FIRST (np-
outer), so the warmup window only needs the small fp8 tiles -> earlier
first matmul and no cold-window DMA starvation.
"""

import math

import numpy as np
import ml_dtypes

H = 2048          # hidden size
I = 1408          # intermediate per routed expert
E = 16            # routed experts
G = 4             # groups
TOPK_GROUP = 2
TOP_K = 6
N_SHARED = 2
SCALE_FACTOR = 2.5
SI = I * N_SHARED  # 2816 shared intermediate
N_CORES = 8
EXP_PER_CORE = E // N_CORES  # 2
P = 128
BF16 = ml_dtypes.bfloat16
FP8 = ml_dtypes.float8_e4m3fn

N8 = 5            # fp8 gate/up I-blocks per expert (of MI=11)
SX = 32.0         # x pre-scale for e4m3
SW = 4096.0       # weight pre-scale for e4m3
SINV = 1.0 / (SX * SW)

_COMPILED = {}  # (T, caps) -> nc
_LAST = {}      # debug/profiling handle for test.py


def _gate_host(hs, gate_weight, bias):
    """numpy replica of reference._gate (verified bit-identical selection)."""
    T = hs.shape[0]
    logits = hs @ gate_weight.T                       # [T, E] fp32
    scores = 1.0 / (1.0 + np.exp(-logits))
    sfc = scores + bias[None, :]
    gs = sfc.reshape(T, G, E // G)
    gsort = np.sort(gs, axis=-1)
    group_scores = gsort[..., -1] + gsort[..., -2]
    group_idx = np.argsort(-group_scores, axis=-1, kind="stable")[:, :TOPK_GROUP]
    gmask = np.zeros((T, G), bool)
    gmask[np.arange(T)[:, None], group_idx] = True
    smask = np.repeat(gmask, E // G, axis=1)
    tmp = np.where(smask, sfc, 0.0)
    topk_idx = np.argsort(-tmp, axis=-1, kind="stable")[:, :TOP_K]
    topk_w = np.take_along_axis(scores, topk_idx, axis=1)
    topk_w = topk_w / (topk_w.sum(-1, keepdims=True) + 1e-20) * SCALE_FACTOR
    return topk_idx.astype(np.int32), topk_w.astype(np.float32)


def _pw(w):
    return ((w + 15) // 16) * 16


def _build(T, caps):
    """Build + compile the SPMD Bass program.

    T    : total tokens; each core's shared slice is TS = T/4 of them
    caps : per expert slot, (C_cap, w): gathered-token capacity and matmul
           free-dim slice width; C_cap = NP_R * 2 * w
    """
    import concourse.mybir as mybir
    import concourse.tile as tile
    from concourse import bacc

    bf = mybir.dt.bfloat16
    f8 = mybir.dt.float8e4
    f32 = mybir.dt.float32
    AF = mybir.ActivationFunctionType
    DR = mybir.MatmulPerfMode.DoubleRow

    KH = H // P        # 16 contraction chunks over H
    KC = KH // 2       # 8 fp8 DoubleRow chunks (K=256 each)
    MI = I // P        # 11 I chunks
    MH = H // P        # 16 output H chunks
    MSI = SI // (2 * P)   # 11 shared-intermediate chunks (half columns)
    TS = T // 4           # 1024 token slice for the shared expert (D=2)
    assert TS == 1024
    for (C_cap, w) in caps:
        assert C_cap % (2 * w) == 0 and w <= 512
    C_tot = sum(C_cap for C_cap, _ in caps)
    slot_base = [sum(C for C, _ in caps[:s]) for s in range(len(caps))]
    w_max = max(w for _, w in caps)
    pws = [_pw(w) for _, w in caps]

    nc = bacc.Bacc("TRN2", target_bir_lowering=False, debug=False,
                   num_devices=N_CORES)
    # x panels are host-packed to [tile, p, kk*cols+c] so each tile loads as
    # one DMA with fat (multi-KB) contiguous per-partition rows
    xs = nc.dram_tensor("xs", [KH // 4, P, 4 * TS], bf, kind="ExternalInput")
    xgs = [
        nc.dram_tensor(f"xg{s}", [(C // (2 * w)) * (KH // 4), P, 4 * 2 * w],
                       bf, kind="ExternalInput")
        for s, (C, w) in enumerate(caps)
    ]
    # fp8 DoubleRow x panels: per np block, 4 tiles of [P, kk=2, i=2, 2*PW]
    # (chunk c=2q+kk contracts h = c*256 + i*128 + p; col = j*PW + n)
    xg8s = [
        nc.dram_tensor(f"xg8{s}", [(C // (2 * w)) * 4, P, 2 * 2 * 2 * pw],
                       f8, kind="ExternalInput")
        for s, ((C, w), pw) in enumerate(zip(caps, pws))
    ]
    # weight panels are pre-tiled on the host to the exact SBUF tile layout
    # bf16 gate/up panels exist only for the MI-N8 bf16 blocks
    wg = nc.dram_tensor("wg", [EXP_PER_CORE * (MI - N8), P, KH * P], bf,
                        kind="ExternalInput")
    wu = nc.dram_tensor("wu", [EXP_PER_CORE * (MI - N8), P, KH * P], bf,
                        kind="ExternalInput")
    # fp8 DoubleRow gate/up panels for the first N8 blocks: [P, c, i, m]
    wg8 = nc.dram_tensor("wg8", [EXP_PER_CORE * N8, P, KC * 2 * P], f8,
                         kind="ExternalInput")
    wu8 = nc.dram_tensor("wu8", [EXP_PER_CORE * N8, P, KC * 2 * P], f8,
                         kind="ExternalInput")
    wd = nc.dram_tensor("wd", [EXP_PER_CORE * MH, P, MI * P], bf,
                        kind="ExternalInput")
    sg = nc.dram_tensor("sg", [MSI, P, KH * P], bf, kind="ExternalInput")
    su = nc.dram_tensor("su", [MSI, P, KH * P], bf, kind="ExternalInput")
    sd = nc.dram_tensor("sd", [MH, P, MSI * P], bf, kind="ExternalInput")
    ye = nc.dram_tensor("ye", [H, C_tot], bf, kind="ExternalOutput")
    ys = nc.dram_tensor("ys", [H, TS], bf, kind="ExternalOutput")

    with tile.TileContext(nc) as tc:
        with (
            # xgp serves both the routed bf16 x quads AND (phase-disjoint,
            # via ring reuse) the shared-expert x tiles — both [128,4,1024]
            tc.tile_pool(name="xgp", bufs=8) as xgp,
            tc.tile_pool(name="x8p", bufs=9) as x8p,   # fp8 DR x tiles
            tc.tile_pool(name="wp", bufs=7) as wp,     # [128,16,128] bf16 w
            tc.tile_pool(name="w8p", bufs=10) as w8p,  # [128,8,2,128] fp8 w
            tc.tile_pool(name="wdp", bufs=2) as wdp,   # [128,11,128] down cols
            tc.tile_pool(name="sdp", bufs=3) as sdp,   # [128,22,128] shared down
            tc.tile_pool(name="itp", bufs=44) as itp,  # [128,512] bf16 inter
            tc.tile_pool(name="tmp", bufs=2) as tmp,   # silu temp
            tc.tile_pool(name="otp", bufs=3) as otp,   # [128,1024] bf16 out
            tc.tile_pool(name="pg", bufs=3, space="PSUM") as pgp,
            tc.tile_pool(name="pu", bufs=3, space="PSUM") as pup,
            tc.tile_pool(name="py", bufs=2, space="PSUM") as pyp,
        ):
            # PE clock warm-up: the HAM gate keeps the PE at 1.2 GHz until
            # ~3.4us of sustained activity, and the first real matmul is
            # data-bound at ~10.6us.  Six dummy matmuls (cold ~430ns each,
            # issuing from ~8.2us) end right as the real stream starts; the
            # real matmuls then extend the busy window so the 2.4 GHz flip
            # lands at ~11.6us instead of ~18.5us.  More dummies delay the
            # real start (measured: 14 pushed it to 13.0us for no net win).
            warm = tmp.tile([P, 512], bf, name="warm", tag="tmp")
            nc.gpsimd.memset(warm[:], 0.0)
            wps = pyp.tile([P, 512], f32, name="warmps", tag="py")
            for _ in range(6):
                nc.tensor.matmul(wps[:], warm[:, :128], warm[:],
                                 start=True, stop=True)

            # Queue discipline: gpsimd carries ONLY data-dependent writes
            # (plus the late sd loads behind them), so no load trigger ever
            # queues behind a write's semaphore wait.  Loads ride
            # scalar+sync.
            def load_w8(s, wait=None, split_first=False, ms=None):
                """fp8 gate/up weight tiles for the N8 fp8 blocks."""
                tiles = {}
                with tc.tile_wait_until(wait, enable=wait is not None):
                    for m in (range(N8) if ms is None else ms):
                        wgt = w8p.tile([P, KC, 2, P], f8, name=f"w8g{s}_{m}",
                                       tag="w8")
                        src = wg8[s * N8 + m]
                        if split_first and m == 0:
                            nc.sync.dma_start(
                                wgt[:, :KC // 2],
                                src[:, :KC // 2 * 2 * P]
                                .rearrange("p (c i m) -> p c i m", i=2, m=P))
                            nc.sync.dma_start(
                                wgt[:, KC // 2:],
                                src[:, KC // 2 * 2 * P:]
                                .rearrange("p (c i m) -> p c i m", i=2, m=P))
                        else:
                            eng = (nc.sync, nc.scalar)[m % 2]
                            eng.dma_start(
                                wgt[:],
                                src.rearrange("p (c i m) -> p c i m",
                                              i=2, m=P))
                        wut = w8p.tile([P, KC, 2, P], f8, name=f"w8u{s}_{m}",
                                       tag="w8")
                        eng = (nc.scalar, nc.sync)[m % 2]
                        eng.dma_start(
                            wut[:],
                            wu8[s * N8 + m]
                            .rearrange("p (c i m) -> p c i m", i=2, m=P))
                        tiles[m] = (wgt, wut)
                return tiles

            def load_x8(s, wait=None, nps=None):
                """fp8 DR x tiles: 4 per np block, each 2 chunks."""
                C_cap, w = caps[s]
                pw = pws[s]
                NP_R = C_cap // (2 * w)
                t8 = {}
                with tc.tile_wait_until(wait, enable=wait is not None):
                    for np_ in (range(NP_R) if nps is None else nps):
                        for q in range(4):
                            if s == 0 and np_ == 0 and q == 0 and wait is None:
                                # warmup: kk-split halves so chain k=0 can
                                # fire after only 0.21MB
                                ta = x8p.tile([P, 1, 2, 2 * pw], f8,
                                              name=f"x8{s}_0_0a", tag="x8")
                                nc.scalar.dma_start(
                                    ta[:],
                                    xg8s[s][0][:, :2 * 2 * pw]
                                    .rearrange("p (kk i c) -> p kk i c",
                                               i=2, c=2 * pw))
                                tb = x8p.tile([P, 1, 2, 2 * pw], f8,
                                              name=f"x8{s}_0_0b", tag="x8")
                                nc.sync.dma_start(
                                    tb[:],
                                    xg8s[s][0][:, 2 * 2 * pw:]
                                    .rearrange("p (kk i c) -> p kk i c",
                                               i=2, c=2 * pw))
                                t8[(np_, q)] = ("split", ta, tb)
                                continue
                            t = x8p.tile([P, 2, 2, 2 * pw], f8,
                                         name=f"x8{s}_{np_}_{q}", tag="x8")
                            eng = (nc.scalar, nc.sync)[q % 2]
                            eng.dma_start(
                                t[:],
                                xg8s[s][np_ * 4 + q]
                                .rearrange("p (kk i c) -> p kk i c",
                                           i=2, c=2 * pw))
                            t8[(np_, q)] = t
                return t8

            def load_x(s, wait=None):
                """bf16 x quads (used by the MI-N8 bf16 blocks)."""
                C_cap, w = caps[s]
                NP_R = C_cap // (2 * w)
                xgt = {}
                with tc.tile_wait_until(wait, enable=wait is not None):
                    for np_ in range(NP_R):
                        for kq in range(KH // 4):
                            t = xgp.tile([P, 4, 1024], bf,
                                         name=f"xg{s}_{np_}_{kq}", tag="x")
                            eng = (nc.scalar, nc.sync)[kq % 2]
                            eng.dma_start(
                                t[:, :, :2 * w],
                                xgs[s][np_ * (KH // 4) + kq]
                                .rearrange("p (kk c) -> p kk c", c=2 * w))
                            xgt[(np_, kq)] = t
                return xgt

            def gate_up(s, xgt, x8t, w8tiles, wwait=None):
                C_cap, w = caps[s]
                pw = pws[s]
                NP_R = C_cap // (2 * w)
                inter = {}
                wtiles = {}

                def wts(m):
                    # bf16 weights for blocks m >= N8, loaded on first use
                    if m not in wtiles:
                        mi = s * (MI - N8) + (m - N8)
                        with tc.tile_wait_until(wwait,
                                                enable=wwait is not None):
                            wgt = wp.tile([P, KH, P], bf, name=f"wgt{s}_{m}",
                                          tag="wp")
                            nc.sync.dma_start(
                                wgt[:],
                                wg[mi].rearrange("p (ko c) -> p ko c", c=P))
                            wut = wp.tile([P, KH, P], bf, name=f"wut{s}_{m}",
                                          tag="wp")
                            nc.sync.dma_start(
                                wut[:],
                                wu[mi].rearrange("p (ko c) -> p ko c", c=P))
                        wtiles[m] = (wgt, wut)
                    return wtiles[m]

                def x8op(np_, k, j):
                    t = x8t[(np_, k // 2)]
                    if isinstance(t, tuple):
                        return t[1 + k % 2][:, 0, :, j * pw:j * pw + w]
                    return t[:, k % 2, :, j * pw:j * pw + w]

                # --- fp8 DoubleRow blocks, np-outer so warmup reuses np0 x ---
                for np_ in range(NP_R):
                    for m in range(N8):
                        wgt, wut = w8tiles[m]
                        for j in range(2):
                            psg = pgp.tile([P, 512], f32,
                                           name=f"psg{s}_{m}_{np_}{j}",
                                           tag="pg")
                            for k in range(KC):
                                nc.tensor.matmul(
                                    psg[:, :w], wgt[:, k, :, :],
                                    x8op(np_, k, j),
                                    start=(k == 0), stop=(k == KC - 1),
                                    perf_mode=DR)
                            st = tmp.tile([P, 512], bf,
                                          name=f"st{s}_{m}_{np_}{j}",
                                          tag="tmp")
                            nc.scalar.activation(st[:, :w], psg[:, :w],
                                                 AF.Silu, scale=SINV)
                            psu = pup.tile([P, 512], f32,
                                           name=f"psu{s}_{m}_{np_}{j}",
                                           tag="pu")
                            for k in range(KC):
                                nc.tensor.matmul(
                                    psu[:, :w], wut[:, k, :, :],
                                    x8op(np_, k, j),
                                    start=(k == 0), stop=(k == KC - 1),
                                    perf_mode=DR)
                            it = itp.tile([P, 512], bf,
                                          name=f"it{s}_{m}_{np_}{j}",
                                          tag="it")
                            # it = silu(g) * (u * SX*SW); the down-proj
                            # columns for this block are host-divided by
                            # SX*SW, so the product is exact
                            nc.vector.tensor_mul(it[:, :w], st[:, :w],
                                                 psu[:, :w])
                            inter[(m, np_, j)] = it

                # --- bf16 blocks (m-outer so weight tiles rotate through
                # the wp ring; all x quads are resident by this point) ---
                for m in range(N8, MI):
                    wgt, wut = wts(m)
                    for np_ in range(NP_R):
                        for j in range(2):
                            psg = pgp.tile([P, 512], f32,
                                           name=f"psg{s}_{m}_{np_}{j}",
                                           tag="pg")
                            for k in range(KH):
                                nc.tensor.matmul(
                                    psg[:, :w], wgt[:, k, :],
                                    xgt[(np_, k // 4)][:, k % 4,
                                                       j * w:(j + 1) * w],
                                    start=(k == 0), stop=(k == KH - 1))
                            st = tmp.tile([P, 512], bf,
                                          name=f"st{s}_{m}_{np_}{j}",
                                          tag="tmp")
                            nc.scalar.activation(st[:, :w], psg[:, :w],
                                                 AF.Silu)
                            psu = pup.tile([P, 512], f32,
                                           name=f"psu{s}_{m}_{np_}{j}",
                                           tag="pu")
                            for k in range(KH):
                                nc.tensor.matmul(
                                    psu[:, :w], wut[:, k, :],
                                    xgt[(np_, k // 4)][:, k % 4,
                                                       j * w:(j + 1) * w],
                                    start=(k == 0), stop=(k == KH - 1))
                            it = itp.tile([P, 512], bf,
                                          name=f"it{s}_{m}_{np_}{j}",
                                          tag="it")
                            nc.vector.tensor_mul(it[:, :w], st[:, :w],
                                                 psu[:, :w])
                            inter[(m, np_, j)] = it
                return inter

            def down(s, inter, wwait=None):
                C_cap, w = caps[s]
                NP_R = C_cap // (2 * w)
                for M in range(MH):
                    wdt = wdp.tile([P, MI, P], bf, name=f"wdt{s}_{M}", tag="wdt")
                    with tc.tile_wait_until(wwait, enable=wwait is not None):
                        nc.sync.dma_start(
                            wdt[:],
                            wd[s * MH + M].rearrange("p (ko c) -> p ko c", c=P))
                    for np_ in range(NP_R):
                        b0 = slot_base[s] + np_ * 2 * w
                        ot = otp.tile([P, 1024], bf,
                                      name=f"ot{s}_{M}_{np_}", tag="ot")
                        for j in range(2):
                            psy = pyp.tile([P, 512], f32,
                                           name=f"psy{s}_{M}_{np_}{j}",
                                           tag="py")
                            for K in range(MI):
                                nc.tensor.matmul(
                                    psy[:, :w], wdt[:, K, :],
                                    inter[(K, np_, j)][:, :w],
                                    start=(K == 0), stop=(K == MI - 1))
                            nc.vector.tensor_copy(
                                ot[:, j * w:(j + 1) * w], psy[:, :w])
                        nc.gpsimd.dma_start(
                            ye[M * P:(M + 1) * P, b0:b0 + 2 * w],
                            ot[:, :2 * w])

            # ---------------- routed experts ----------------
            # tile_wait_until floors (compile-time scheduler hints, in ms)
            # keep non-urgent loads out of the warmup window so slot 0's
            # fp8 stream owns the DMA bandwidth early on
            # warmup DMA order: m0's fp8 weights, np0's fp8 x, the rest of
            # the fp8 weights, np1's fp8 x — the first chains' data never
            # queues behind bytes they don't need yet
            w8_0 = load_w8(0, split_first=True, ms=[0])
            x8_0 = load_x8(0, nps=[0])
            w8_0.update(load_w8(0, ms=list(range(1, N8))))
            x8_0.update(load_x8(0, nps=[1]))
            xgt0 = load_x(0, wait=0.012)
            w8_1 = load_w8(1, wait=0.16)
            x8_1 = load_x8(1, wait=0.18)
            inter0 = gate_up(0, xgt0, x8_0, w8_0, wwait=0.015)
            xgt1 = load_x(1, wait=0.22)
            down(0, inter0, wwait=0.10)
            inter1 = gate_up(1, xgt1, x8_1, w8_1, wwait=0.26)
            down(1, inter1, wwait=0.40)

            # ------- shared expert (D=2: half columns x 1024 tokens) -------
            xst = []
            with tc.tile_wait_until(0.42):
                for kq in range(KH // 4):
                    t = xgp.tile([P, 4, TS], bf, name=f"xs{kq}", tag="x")
                    nc.scalar.dma_start(
                        t[:], xs[kq].rearrange("p (kk c) -> p kk c", c=TS))
                    xst.append(t)
            sint = {}
            for m in range(MSI):
                sgt = wp.tile([P, KH, P], bf, name=f"sgt{m}", tag="wp")
                nc.sync.dma_start(
                    sgt[:], sg[m].rearrange("p (ko c) -> p ko c", c=P))
                sut = wp.tile([P, KH, P], bf, name=f"sut{m}", tag="wp")
                nc.sync.dma_start(
                    sut[:], su[m].rearrange("p (ko c) -> p ko c", c=P))
                for j in range(2):
                    psg = pgp.tile([P, 512], f32, name=f"psgs{m}{j}", tag="pg")
                    for k in range(KH):
                        nc.tensor.matmul(
                            psg[:], sgt[:, k, :],
                            xst[k // 4][:, k % 4, j * 512:(j + 1) * 512],
                            start=(k == 0), stop=(k == KH - 1))
                    st = tmp.tile([P, 512], bf, name=f"sts{m}{j}", tag="tmp")
                    nc.scalar.activation(st[:], psg[:], AF.Silu)
                    psu = pup.tile([P, 512], f32, name=f"psus{m}{j}", tag="pu")
                    for k in range(KH):
                        nc.tensor.matmul(
                            psu[:], sut[:, k, :],
                            xst[k // 4][:, k % 4, j * 512:(j + 1) * 512],
                            start=(k == 0), stop=(k == KH - 1))
                    it = itp.tile([P, 512], bf, name=f"si{m}{j}", tag="it")
                    nc.vector.tensor_mul(it[:], st[:], psu[:])
                    sint[(m, j)] = it
            for M in range(MH):
                sdt = sdp.tile([P, MSI, P], bf, name=f"sdt{M}", tag="sdt")
                with tc.tile_wait_until(0.60):
                    nc.gpsimd.dma_start(
                        sdt[:], sd[M].rearrange("p (ko c) -> p ko c", c=P))
                ot = otp.tile([P, 1024], bf, name=f"ots{M}", tag="ot")
                # last M drains in smaller chunks so the final cast+DMA
                # tail after the last matmul is shorter
                chunks = ([(0, 512), (512, 512)] if M < MH - 1
                          else [(0, 512), (512, 256), (768, 256)])
                for ci, (c0, cw) in enumerate(chunks):
                    j = min(c0 // 512, 1)
                    psy = pyp.tile([P, 512], f32, name=f"psys{M}_{ci}",
                                   tag="py")
                    for K in range(MSI):
                        nc.tensor.matmul(
                            psy[:, :cw], sdt[:, K, :],
                            sint[(K, j)][:, c0 - j * 512:c0 - j * 512 + cw],
                            start=(K == 0), stop=(K == MSI - 1))
                    nc.vector.tensor_copy(ot[:, c0:c0 + cw], psy[:, :cw])
                    nc.scalar.dma_start(
                        ys[M * P:(M + 1) * P, c0:c0 + cw],
                        ot[:, c0:c0 + cw])

    nc.compile()
    return nc


def _get_compiled(T, caps):
    key = (T, tuple(caps))
    if key not in _COMPILED:
        _COMPILED[key] = _build(T, caps)
    return _COMPILED[key]


def _cap_for(maxc):
    maxc = max(int(maxc), 64)
    np_r = max(2, math.ceil(maxc / 2048))
    w = min(512, 2 * math.ceil(maxc / (np_r * 2 * 2)))
    C_cap = np_r * 2 * w
    assert C_cap >= maxc
    return C_cap, w


def kernel(hidden_states, gate_weight, e_score_correction_bias,
           gate_proj, up_proj, down_proj,
           shared_gate_w, shared_up_w, shared_down_w):
    from concourse.bass_utils import run_bass_kernel_spmd

    hs = np.asarray(hidden_states, dtype=np.float32)
    B, S, Hh = hs.shape
    assert Hh == H
    hsf = np.ascontiguousarray(hs.reshape(-1, H))
    T = hsf.shape[0]
    TS = T // 4        # shared-expert token slice (D=2 hybrid shard)
    gate_weight = np.asarray(gate_weight, np.float32)
    bias = np.asarray(e_score_correction_bias, np.float32)
    gate_proj = np.asarray(gate_proj, np.float32)
    up_proj = np.asarray(up_proj, np.float32)
    down_proj = np.asarray(down_proj, np.float32)
    shared_gate_w = np.asarray(shared_gate_w, np.float32)
    shared_up_w = np.asarray(shared_up_w, np.float32)
    shared_down_w = np.asarray(shared_down_w, np.float32)

    # ---- routing on host ----
    topk_idx, topk_w = _gate_host(hsf, gate_weight, bias)
    comb = np.zeros((T, E), np.float32)
    np.add.at(comb, (np.arange(T)[:, None], topk_idx), topk_w)
    sel = np.zeros((T, E), bool)
    sel[np.arange(T)[:, None], topk_idx] = True
    idx_e = [np.nonzero(sel[:, e])[0] for e in range(E)]
    counts = np.array([len(ix) for ix in idx_e])

    # assign experts to (core, slot): slot 0 gets the 8 largest, slot 1 the
    # 8 smallest, so each slot's capacity (uniform across cores under SPMD)
    # hugs its own max count
    order = np.argsort(-counts, kind="stable")
    assign = np.zeros((N_CORES, EXP_PER_CORE), np.int64)
    for c in range(N_CORES):
        assign[c, 0] = order[c]
        assign[c, 1] = order[2 * N_CORES - 1 - c]
    caps = [
        _cap_for(counts[assign[:, 0]].max()),
        _cap_for(counts[assign[:, 1]].max()),
    ]
    slot_base = [0, caps[0][0]]
    C_tot = caps[0][0] + caps[1][0]

    # ---- host-side dispatch (shard + transpose + bf16/fp8 cast) ----
    xsT = np.ascontiguousarray(hsf.T)                       # [H, T] fp32
    xsTb = xsT.astype(BF16)
    xsT8 = np.clip(xsT * SX, -240, 240).astype(FP8)

    MI, MH, MSI, KH = I // P, H // P, SI // (2 * P), H // P
    KC = KH // 2
    SIH = SI // 2      # 1408 shared-intermediate columns per group
    NF8 = N8 * P       # fp8 I-rows per expert

    def tile_gu(wmat, nm):  # [I', H] -> [nm, P, KH*P] : (m, p_h, ko_h*P + c_i)
        return np.ascontiguousarray(
            wmat.reshape(nm, P, KH, P).transpose(0, 3, 2, 1)
        ).reshape(nm, P, KH * P).astype(BF16)

    def tile_gu8(wmat):  # [NF8, H] fp32 -> [N8, P, KC*2*P] fp8 DR layout
        q = np.clip(wmat * SW, -240, 240).astype(FP8)
        # [m, r, c, i, p] -> [m, p, c, i, r]
        a = q.reshape(N8, P, KC, 2, P).transpose(0, 4, 2, 3, 1)
        return np.ascontiguousarray(a).reshape(N8, P, KC * 2 * P)

    def tile_dn(wmat, nk):  # [H, I'] -> [MH, P, nk*P] : (M, p_i, Ko_i*P + c_h)
        return np.ascontiguousarray(
            wmat.reshape(MH, P, nk, P).transpose(0, 3, 2, 1)
        ).reshape(MH, P, nk * P).astype(BF16)

    # shared weights: two column groups (cores 0-3 and 4-7); each core also
    # takes a 1024-token slice, so the shared output is a 2-way partial sum
    sg_g = [tile_gu(shared_gate_w[g * SIH:(g + 1) * SIH], MSI) for g in (0, 1)]
    su_g = [tile_gu(shared_up_w[g * SIH:(g + 1) * SIH], MSI) for g in (0, 1)]
    sd_g = [tile_dn(shared_down_w[:, g * SIH:(g + 1) * SIH], MSI)
            for g in (0, 1)]

    def pack_panels(xmat, NP, cols):
        # [H, NP*cols] -> [NP*4, P, 4*cols], tile np*4+kq holds h rows
        # (kq*4+kk)*128+p and cols [np*cols + c]
        a = xmat.reshape(4, 4, P, NP, cols)          # [kq, kk, p, np, c]
        return np.ascontiguousarray(
            a.transpose(3, 0, 2, 1, 4).reshape(NP * 4, P, 4 * cols)
        )

    def pack_panels8(x8, NP, w, pw):
        # [H, NP*2w] fp8 -> [NP*4, P, 2*2*2pw]: tile np*4+q holds chunks
        # c = 2q+kk (h = c*256 + i*128 + p), cols j*pw + n (n < w)
        a = x8.reshape(8, 2, P, NP, 2, w)            # [c, i, p, np, j, n]
        out = np.zeros((NP, 4, 2, P, 2, 2, pw), FP8)  # [np,q,kk,p,i,j,col]
        out[..., :w] = a.transpose(3, 0, 2, 1, 4, 5).reshape(
            NP, 4, 2, P, 2, 2, w)
        return np.ascontiguousarray(
            out.transpose(0, 1, 3, 2, 4, 5, 6).reshape(NP * 4, P, 2 * 2 * 2 * pw))

    in_maps = []
    for c in range(N_CORES):
        e0, e1 = assign[c]
        xg_pk, xg8_pk = [], []
        for sslot, e in enumerate((e0, e1)):
            C_cap, w = caps[sslot]
            pw = _pw(w)
            NP_R = C_cap // (2 * w)
            xg_c = np.zeros((H, C_cap), BF16)
            xg_c[:, :counts[e]] = xsTb[:, idx_e[e]]
            xg_pk.append(pack_panels(xg_c, NP_R, 2 * w))
            xg8_c = np.zeros((H, C_cap), FP8)
            xg8_c[:, :counts[e]] = xsT8[:, idx_e[e]]
            xg8_pk.append(pack_panels8(xg8_c, NP_R, w, pw))
        wg_c = np.concatenate([tile_gu(gate_proj[e][NF8:], MI - N8)
                               for e in (e0, e1)])
        wu_c = np.concatenate([tile_gu(up_proj[e][NF8:], MI - N8)
                               for e in (e0, e1)])
        wg8_c = np.concatenate([tile_gu8(gate_proj[e][:NF8])
                                for e in (e0, e1)])
        wu8_c = np.concatenate([tile_gu8(up_proj[e][:NF8])
                                for e in (e0, e1)])
        wd_list = []
        for e in (e0, e1):
            dpe = down_proj[e].copy()
            dpe[:, :NF8] *= SINV      # descale for fp8 blocks' inter
            wd_list.append(tile_dn(dpe, MI))
        wd_c = np.concatenate(wd_list)
        g, ts = c // 4, c % 4
        in_maps.append({
            "xs": pack_panels(
                np.ascontiguousarray(xsTb[:, ts * TS:(ts + 1) * TS]), 1, TS),
            "xg0": xg_pk[0], "xg1": xg_pk[1],
            "xg80": xg8_pk[0], "xg81": xg8_pk[1],
            "wg": wg_c, "wu": wu_c, "wg8": wg8_c, "wu8": wu8_c, "wd": wd_c,
            "sg": sg_g[g], "su": su_g[g], "sd": sd_g[g],
        })

    nc = _get_compiled(T, caps)

    def _run_and_combine():
        results = run_bass_kernel_spmd(nc, in_maps,
                                       core_ids=list(range(N_CORES)))
        outT = np.zeros((H, T), np.float32)
        for c in range(N_CORES):
            ts = c % 4
            outT[:, ts * TS:(ts + 1) * TS] += \
                results.results[c]["ys"].astype(np.float32)
        for c in range(N_CORES):
            ye = results.results[c]["ye"].astype(np.float32)
            for sslot in range(EXP_PER_CORE):
                e = assign[c, sslot]
                cnt = counts[e]
                if cnt == 0:
                    continue
                b0 = slot_base[sslot]
                we = comb[idx_e[e], e]
                outT[:, idx_e[e]] += ye[:, b0:b0 + cnt] * we[None, :]
        return results, outT

    def _spot_err(outT):
        # exact fp32 recompute of a few tokens; the hybrid device path is
        # within ~3e-2 of this, a corrupted pass is off by orders of
        # magnitude
        errs = []
        for t in (0, T // 3, T // 2, T - 1):
            x = hsf[t]
            acc = np.zeros(H, np.float32)
            for e in topk_idx[t]:
                g = gate_proj[e] @ x
                u = up_proj[e] @ x
                inter = g / (1.0 + np.exp(-g)) * u
                acc += comb[t, e] * (down_proj[e] @ inter)
            sg_ = shared_gate_w @ x
            su_ = shared_up_w @ x
            acc += shared_down_w @ (sg_ / (1.0 + np.exp(-sg_)) * su_)
            errs.append(np.linalg.norm(outT[:, t] - acc)
                        / (np.linalg.norm(acc) + 1e-20))
        return max(errs)

    results, outT = _run_and_combine()
    if _spot_err(outT) > 0.2:   # transient device fault: retry once
        results, outT = _run_and_combine()

    _LAST.clear()
    _LAST.update(nc=nc, in_maps=in_maps, results=results, caps=caps)

    return np.ascontiguousarray(outT.T).reshape(B, S, H).astype(np.float32)


# revision 36
# speedup vs baseline: 1.0147x; 1.0122x over previous
"""MoE routing kernel for Trainium2 (8 NeuronCores, SPMD expert-parallel).

Contract: kernel(**full_inputs) -> full output [B, S, H] float32.

Strategy
--------
- Host: compute the (tiny) gate + group-topk routing in numpy (bit-identical
  selection to the jax reference), build the per-(token,expert) combine
  weights, and dispatch: gather each expert's tokens into a padded,
  transposed buffer.  This is the "all-to-all by topk_idx" of the
  sharding hint, done at input-sharding time.
- Device (SPMD over 8 cores): core c holds experts (2c, 2c+1) and a 1/8
  TOKEN-slice of the shared expert.  The routed phase runs first: the
  SwiGLU MLP for its two experts over their gathered tokens (unweighted).
  The shared phase runs last over the core's 512-token slice with the FULL
  shared weights.
- FP8 hybrid: the first N8 (of 11) I-blocks of every routed expert's
  gate/up projections run as fp8-e4m3 DoubleRow matmuls (K=256 per
  instruction -> 2x PE rate); the rest and the whole down/shared path
  stay bf16.  Power-of-2 pre-scales (x*32, W*4096) keep e4m3 out of
  subnormals; the descale folds exactly into the silu activation's scale
  and into host-prescaled down-proj columns, so no extra device work.
  Measured end-to-end rel err ~1.86e-2 at N8=5 (gate 2e-2).
- Host: scale per-expert outputs by routing weights, scatter-add over
  token indices, place each core's shared token-slice, transpose back.

All bf16 matmuls accumulate fp32 in PSUM.  Weight panels are pre-tiled on
the host into the exact SBUF tile layout so each streams from HBM exactly
once as a contiguous per-partition DMA.  The fp8 blocks run FIRST (np-
outer), so the warmup window only needs the small fp8 tiles -> earlier
first matmul and no cold-window DMA starvation.
"""

import math

import numpy as np
import ml_dtypes

H = 2048          # hidden size
I = 1408          # intermediate per routed expert
E = 16            # routed experts
G = 4             # groups
TOPK_GROUP = 2
TOP_K = 6
N_SHARED = 2
SCALE_FACTOR = 2.5
SI = I * N_SHARED  # 2816 shared intermediate
N_CORES = 8
EXP_PER_CORE = E // N_CORES  # 2
P = 128
BF16 = ml_dtypes.bfloat16
FP8 = ml_dtypes.float8_e4m3fn

N8S = (5, 6)      # fp8 gate/up I-blocks per slot's expert (of MI=11):
                  # slot1 (8 smallest experts) carries the extra block;
                  # simulated end-to-end rel err 1.9448e-2 (gate 2e-2)
SX = 32.0         # x pre-scale for e4m3
SW = 4096.0       # weight pre-scale for e4m3
SINV = 1.0 / (SX * SW)

_COMPILED = {}  # (T, caps) -> nc
_LAST = {}      # debug/profiling handle for test.py


def _gate_host(hs, gate_weight, bias):
    """numpy replica of reference._gate (verified bit-identical selection)."""
    T = hs.shape[0]
    logits = hs @ gate_weight.T                       # [T, E] fp32
    scores = 1.0 / (1.0 + np.exp(-logits))
    sfc = scores + bias[None, :]
    gs = sfc.reshape(T, G, E // G)
    gsort = np.sort(gs, axis=-1)
    group_scores = gsort[..., -1] + gsort[..., -2]
    group_idx = np.argsort(-group_scores, axis=-1, kind="stable")[:, :TOPK_GROUP]
    gmask = np.zeros((T, G), bool)
    gmask[np.arange(T)[:, None], group_idx] = True
    smask = np.repeat(gmask, E // G, axis=1)
    tmp = np.where(smask, sfc, 0.0)
    topk_idx = np.argsort(-tmp, axis=-1, kind="stable")[:, :TOP_K]
    topk_w = np.take_along_axis(scores, topk_idx, axis=1)
    topk_w = topk_w / (topk_w.sum(-1, keepdims=True) + 1e-20) * SCALE_FACTOR
    return topk_idx.astype(np.int32), topk_w.astype(np.float32)


def _pw(w):
    return ((w + 15) // 16) * 16


def _build(T, caps):
    """Build + compile the SPMD Bass program.

    T    : total tokens; each core's shared slice is TS = T/4 of them
    caps : per expert slot, (C_cap, w): gathered-token capacity and matmul
           free-dim slice width; C_cap = NP_R * 2 * w
    """
    import concourse.mybir as mybir
    import concourse.tile as tile
    from concourse import bacc

    bf = mybir.dt.bfloat16
    f8 = mybir.dt.float8e4
    f32 = mybir.dt.float32
    AF = mybir.ActivationFunctionType
    DR = mybir.MatmulPerfMode.DoubleRow

    KH = H // P        # 16 contraction chunks over H
    KC = KH // 2       # 8 fp8 DoubleRow chunks (K=256 each)
    MI = I // P        # 11 I chunks
    MH = H // P        # 16 output H chunks
    MSI = SI // (2 * P)   # 11 shared-intermediate chunks (half columns)
    TS = T // 4           # 1024 token slice for the shared expert (D=2)
    assert TS == 1024
    for (C_cap, w) in caps:
        assert C_cap % (2 * w) == 0 and w <= 512
    C_tot = sum(C_cap for C_cap, _ in caps)
    slot_base = [sum(C for C, _ in caps[:s]) for s in range(len(caps))]
    w_max = max(w for _, w in caps)
    pws = [_pw(w) for _, w in caps]

    nc = bacc.Bacc("TRN2", target_bir_lowering=False, debug=False,
                   num_devices=N_CORES)
    # x panels are host-packed to [tile, p, kk*cols+c] so each tile loads as
    # one DMA with fat (multi-KB) contiguous per-partition rows
    xs = nc.dram_tensor("xs", [KH // 4, P, 4 * TS], bf, kind="ExternalInput")
    xgs = [
        nc.dram_tensor(f"xg{s}", [(C // (2 * w)) * (KH // 4), P, 4 * 2 * w],
                       bf, kind="ExternalInput")
        for s, (C, w) in enumerate(caps)
    ]
    # fp8 DoubleRow x panels: per np block, 4 tiles of [P, kk=2, i=2, 2*PW]
    # (chunk c=2q+kk contracts h = c*256 + i*128 + p; col = j*PW + n)
    xg8s = [
        nc.dram_tensor(f"xg8{s}", [(C // (2 * w)) * 4, P, 2 * 2 * 2 * pw],
                       f8, kind="ExternalInput")
        for s, ((C, w), pw) in enumerate(zip(caps, pws))
    ]
    # weight panels are pre-tiled on the host to the exact SBUF tile layout
    # bf16 gate/up panels exist only for the MI-N8S[s] bf16 blocks
    W8O = [0, N8S[0]]                 # per-slot offsets into wg8/wu8
    WBO = [0, MI - N8S[0]]            # per-slot offsets into wg/wu
    wg = nc.dram_tensor("wg", [sum(MI - n for n in N8S), P, KH * P], bf,
                        kind="ExternalInput")
    wu = nc.dram_tensor("wu", [sum(MI - n for n in N8S), P, KH * P], bf,
                        kind="ExternalInput")
    # fp8 DoubleRow gate/up panels for the first N8S[s] blocks: [P, c, i, m]
    wg8 = nc.dram_tensor("wg8", [sum(N8S), P, KC * 2 * P], f8,
                         kind="ExternalInput")
    wu8 = nc.dram_tensor("wu8", [sum(N8S), P, KC * 2 * P], f8,
                         kind="ExternalInput")
    wd = nc.dram_tensor("wd", [EXP_PER_CORE * MH, P, MI * P], bf,
                        kind="ExternalInput")
    sg = nc.dram_tensor("sg", [MSI, P, KH * P], bf, kind="ExternalInput")
    su = nc.dram_tensor("su", [MSI, P, KH * P], bf, kind="ExternalInput")
    sd = nc.dram_tensor("sd", [MH, P, MSI * P], bf, kind="ExternalInput")
    ye = nc.dram_tensor("ye", [H, C_tot], bf, kind="ExternalOutput")
    ys = nc.dram_tensor("ys", [H, TS], bf, kind="ExternalOutput")

    with tile.TileContext(nc) as tc:
        with (
            # xgp serves both the routed bf16 x quads AND (phase-disjoint,
            # via ring reuse) the shared-expert x tiles — both [128,4,1024]
            tc.tile_pool(name="xgp", bufs=8) as xgp,
            tc.tile_pool(name="x8p", bufs=9) as x8p,   # fp8 DR x tiles
            tc.tile_pool(name="wp", bufs=6) as wp,     # [128,16,128] bf16 w
            tc.tile_pool(name="w8p", bufs=12) as w8p,  # [128,8,2,128] fp8 w
            tc.tile_pool(name="wdp", bufs=2) as wdp,   # [128,11,128] down cols
            tc.tile_pool(name="sdp", bufs=3) as sdp,   # [128,22,128] shared down
            tc.tile_pool(name="itp", bufs=44) as itp,  # [128,512] bf16 inter
            tc.tile_pool(name="tmp", bufs=2) as tmp,   # silu temp
            tc.tile_pool(name="otp", bufs=3) as otp,   # [128,1024] bf16 out
            tc.tile_pool(name="pg", bufs=3, space="PSUM") as pgp,
            tc.tile_pool(name="pu", bufs=3, space="PSUM") as pup,
            tc.tile_pool(name="py", bufs=2, space="PSUM") as pyp,
        ):
            # PE clock warm-up: the HAM gate keeps the PE at 1.2 GHz until
            # ~3.4us of sustained activity, and the first real matmul is
            # data-bound at ~10.6us.  Six dummy matmuls (cold ~430ns each,
            # issuing from ~8.2us) end right as the real stream starts; the
            # real matmuls then extend the busy window so the 2.4 GHz flip
            # lands at ~11.6us instead of ~18.5us.  More dummies delay the
            # real start (measured: 14 pushed it to 13.0us for no net win).
            warm = tmp.tile([P, 512], bf, name="warm", tag="tmp")
            nc.gpsimd.memset(warm[:], 0.0)
            wps = pyp.tile([P, 512], f32, name="warmps", tag="py")
            for _ in range(6):
                nc.tensor.matmul(wps[:], warm[:, :128], warm[:],
                                 start=True, stop=True)

            # Queue discipline: gpsimd carries ONLY data-dependent writes
            # (plus the late sd loads behind them), so no load trigger ever
            # queues behind a write's semaphore wait.  Loads ride
            # scalar+sync.
            def load_w8(s, wait=None, split_first=False, ms=None):
                """fp8 gate/up weight tiles for the N8 fp8 blocks."""
                tiles = {}
                with tc.tile_wait_until(wait, enable=wait is not None):
                    for m in (range(N8S[s]) if ms is None else ms):
                        wgt = w8p.tile([P, KC, 2, P], f8, name=f"w8g{s}_{m}",
                                       tag="w8")
                        src = wg8[W8O[s] + m]
                        if split_first and m == 0:
                            nc.sync.dma_start(
                                wgt[:, :KC // 2],
                                src[:, :KC // 2 * 2 * P]
                                .rearrange("p (c i m) -> p c i m", i=2, m=P))
                            nc.sync.dma_start(
                                wgt[:, KC // 2:],
                                src[:, KC // 2 * 2 * P:]
                                .rearrange("p (c i m) -> p c i m", i=2, m=P))
                        else:
                            eng = (nc.sync, nc.scalar)[m % 2]
                            eng.dma_start(
                                wgt[:],
                                src.rearrange("p (c i m) -> p c i m",
                                              i=2, m=P))
                        wut = w8p.tile([P, KC, 2, P], f8, name=f"w8u{s}_{m}",
                                       tag="w8")
                        eng = (nc.scalar, nc.sync)[m % 2]
                        eng.dma_start(
                            wut[:],
                            wu8[W8O[s] + m]
                            .rearrange("p (c i m) -> p c i m", i=2, m=P))
                        tiles[m] = (wgt, wut)
                return tiles

            def load_x8(s, wait=None, nps=None):
                """fp8 DR x tiles: 4 per np block, each 2 chunks."""
                C_cap, w = caps[s]
                pw = pws[s]
                NP_R = C_cap // (2 * w)
                t8 = {}
                with tc.tile_wait_until(wait, enable=wait is not None):
                    for np_ in (range(NP_R) if nps is None else nps):
                        for q in range(4):
                            if s == 0 and np_ == 0 and q == 0 and wait is None:
                                # warmup: kk-split halves so chain k=0 can
                                # fire after only 0.21MB
                                ta = x8p.tile([P, 1, 2, 2 * pw], f8,
                                              name=f"x8{s}_0_0a", tag="x8")
                                nc.scalar.dma_start(
                                    ta[:],
                                    xg8s[s][0][:, :2 * 2 * pw]
                                    .rearrange("p (kk i c) -> p kk i c",
                                               i=2, c=2 * pw))
                                tb = x8p.tile([P, 1, 2, 2 * pw], f8,
                                              name=f"x8{s}_0_0b", tag="x8")
                                nc.sync.dma_start(
                                    tb[:],
                                    xg8s[s][0][:, 2 * 2 * pw:]
                                    .rearrange("p (kk i c) -> p kk i c",
                                               i=2, c=2 * pw))
                                t8[(np_, q)] = ("split", ta, tb)
                                continue
                            t = x8p.tile([P, 2, 2, 2 * pw], f8,
                                         name=f"x8{s}_{np_}_{q}", tag="x8")
                            eng = (nc.scalar, nc.sync)[q % 2]
                            eng.dma_start(
                                t[:],
                                xg8s[s][np_ * 4 + q]
                                .rearrange("p (kk i c) -> p kk i c",
                                           i=2, c=2 * pw))
                            t8[(np_, q)] = t
                return t8

            def load_x(s, wait=None):
                """bf16 x quads (used by the MI-N8 bf16 blocks)."""
                C_cap, w = caps[s]
                NP_R = C_cap // (2 * w)
                xgt = {}
                with tc.tile_wait_until(wait, enable=wait is not None):
                    for np_ in range(NP_R):
                        for kq in range(KH // 4):
                            t = xgp.tile([P, 4, 1024], bf,
                                         name=f"xg{s}_{np_}_{kq}", tag="x")
                            eng = (nc.scalar, nc.sync)[kq % 2]
                            eng.dma_start(
                                t[:, :, :2 * w],
                                xgs[s][np_ * (KH // 4) + kq]
                                .rearrange("p (kk c) -> p kk c", c=2 * w))
                            xgt[(np_, kq)] = t
                return xgt

            def gate_up(s, xgt, x8t, w8tiles, wwait=None):
                C_cap, w = caps[s]
                pw = pws[s]
                NP_R = C_cap // (2 * w)
                inter = {}
                wtiles = {}

                def wts(m):
                    # bf16 weights for blocks m >= N8S[s], loaded on first use
                    if m not in wtiles:
                        mi = WBO[s] + (m - N8S[s])
                        with tc.tile_wait_until(wwait,
                                                enable=wwait is not None):
                            wgt = wp.tile([P, KH, P], bf, name=f"wgt{s}_{m}",
                                          tag="wp")
                            nc.sync.dma_start(
                                wgt[:],
                                wg[mi].rearrange("p (ko c) -> p ko c", c=P))
                            wut = wp.tile([P, KH, P], bf, name=f"wut{s}_{m}",
                                          tag="wp")
                            nc.sync.dma_start(
                                wut[:],
                                wu[mi].rearrange("p (ko c) -> p ko c", c=P))
                        wtiles[m] = (wgt, wut)
                    return wtiles[m]

                def x8op(np_, k, j):
                    t = x8t[(np_, k // 2)]
                    if isinstance(t, tuple):
                        return t[1 + k % 2][:, 0, :, j * pw:j * pw + w]
                    return t[:, k % 2, :, j * pw:j * pw + w]

                # --- fp8 DoubleRow blocks, np-outer so warmup reuses np0 x ---
                for np_ in range(NP_R):
                    for m in range(N8S[s]):
                        wgt, wut = w8tiles[m]
                        for j in range(2):
                            psg = pgp.tile([P, 512], f32,
                                           name=f"psg{s}_{m}_{np_}{j}",
                                           tag="pg")
                            for k in range(KC):
                                nc.tensor.matmul(
                                    psg[:, :w], wgt[:, k, :, :],
                                    x8op(np_, k, j),
                                    start=(k == 0), stop=(k == KC - 1),
                                    perf_mode=DR)
                            st = tmp.tile([P, 512], bf,
                                          name=f"st{s}_{m}_{np_}{j}",
                                          tag="tmp")
                            nc.scalar.activation(st[:, :w], psg[:, :w],
                                                 AF.Silu, scale=SINV)
                            psu = pup.tile([P, 512], f32,
                                           name=f"psu{s}_{m}_{np_}{j}",
                                           tag="pu")
                            for k in range(KC):
                                nc.tensor.matmul(
                                    psu[:, :w], wut[:, k, :, :],
                                    x8op(np_, k, j),
                                    start=(k == 0), stop=(k == KC - 1),
                                    perf_mode=DR)
                            it = itp.tile([P, 512], bf,
                                          name=f"it{s}_{m}_{np_}{j}",
                                          tag="it")
                            # it = silu(g) * (u * SX*SW); the down-proj
                            # columns for this block are host-divided by
                            # SX*SW, so the product is exact
                            nc.vector.tensor_mul(it[:, :w], st[:, :w],
                                                 psu[:, :w])
                            inter[(m, np_, j)] = it

                # --- bf16 blocks (m-outer so weight tiles rotate through
                # the wp ring; all x quads are resident by this point) ---
                for m in range(N8S[s], MI):
                    wgt, wut = wts(m)
                    for np_ in range(NP_R):
                        for j in range(2):
                            psg = pgp.tile([P, 512], f32,
                                           name=f"psg{s}_{m}_{np_}{j}",
                                           tag="pg")
                            for k in range(KH):
                                nc.tensor.matmul(
                                    psg[:, :w], wgt[:, k, :],
                                    xgt[(np_, k // 4)][:, k % 4,
                                                       j * w:(j + 1) * w],
                                    start=(k == 0), stop=(k == KH - 1))
                            st = tmp.tile([P, 512], bf,
                                          name=f"st{s}_{m}_{np_}{j}",
                                          tag="tmp")
                            nc.scalar.activation(st[:, :w], psg[:, :w],
                                                 AF.Silu)
                            psu = pup.tile([P, 512], f32,
                                           name=f"psu{s}_{m}_{np_}{j}",
                                           tag="pu")
                            for k in range(KH):
                                nc.tensor.matmul(
                                    psu[:, :w], wut[:, k, :],
                                    xgt[(np_, k // 4)][:, k % 4,
                                                       j * w:(j + 1) * w],
                                    start=(k == 0), stop=(k == KH - 1))
                            it = itp.tile([P, 512], bf,
                                          name=f"it{s}_{m}_{np_}{j}",
                                          tag="it")
                            nc.vector.tensor_mul(it[:, :w], st[:, :w],
                                                 psu[:, :w])
                            inter[(m, np_, j)] = it
                return inter

            def down(s, inter, wwait=None):
                C_cap, w = caps[s]
                NP_R = C_cap // (2 * w)
                for M in range(MH):
                    wdt = wdp.tile([P, MI, P], bf, name=f"wdt{s}_{M}", tag="wdt")
                    with tc.tile_wait_until(wwait, enable=wwait is not None):
                        nc.sync.dma_start(
                            wdt[:],
                            wd[s * MH + M].rearrange("p (ko c) -> p ko c", c=P))
                    for np_ in range(NP_R):
                        b0 = slot_base[s] + np_ * 2 * w
                        ot = otp.tile([P, 1024], bf,
                                      name=f"ot{s}_{M}_{np_}", tag="ot")
                        for j in range(2):
                            psy = pyp.tile([P, 512], f32,
                                           name=f"psy{s}_{M}_{np_}{j}",
                                           tag="py")
                            for K in range(MI):
                                nc.tensor.matmul(
                                    psy[:, :w], wdt[:, K, :],
                                    inter[(K, np_, j)][:, :w],
                                    start=(K == 0), stop=(K == MI - 1))
                            nc.vector.tensor_copy(
                                ot[:, j * w:(j + 1) * w], psy[:, :w])
                        nc.gpsimd.dma_start(
                            ye[M * P:(M + 1) * P, b0:b0 + 2 * w],
                            ot[:, :2 * w])

            # ---------------- routed experts ----------------
            # tile_wait_until floors (compile-time scheduler hints, in ms)
            # keep non-urgent loads out of the warmup window so slot 0's
            # fp8 stream owns the DMA bandwidth early on
            # warmup DMA order: m0's fp8 weights, np0's fp8 x, the rest of
            # the fp8 weights, np1's fp8 x — the first chains' data never
            # queues behind bytes they don't need yet
            w8_0 = load_w8(0, split_first=True, ms=[0])
            x8_0 = load_x8(0, nps=[0])
            w8_0.update(load_w8(0, ms=list(range(1, N8S[0]))))
            x8_0.update(load_x8(0, nps=[1]))
            xgt0 = load_x(0, wait=0.012)
            w8_1 = load_w8(1, wait=0.16)
            x8_1 = load_x8(1, wait=0.18)
            inter0 = gate_up(0, xgt0, x8_0, w8_0, wwait=0.015)
            xgt1 = load_x(1, wait=0.22)
            down(0, inter0, wwait=0.10)
            inter1 = gate_up(1, xgt1, x8_1, w8_1, wwait=0.26)
            down(1, inter1, wwait=0.40)

            # ------- shared expert (D=2: half columns x 1024 tokens) -------
            xst = []
            with tc.tile_wait_until(0.42):
                for kq in range(KH // 4):
                    t = xgp.tile([P, 4, TS], bf, name=f"xs{kq}", tag="x")
                    nc.scalar.dma_start(
                        t[:], xs[kq].rearrange("p (kk c) -> p kk c", c=TS))
                    xst.append(t)
            sint = {}
            for m in range(MSI):
                sgt = wp.tile([P, KH, P], bf, name=f"sgt{m}", tag="wp")
                nc.sync.dma_start(
                    sgt[:], sg[m].rearrange("p (ko c) -> p ko c", c=P))
                sut = wp.tile([P, KH, P], bf, name=f"sut{m}", tag="wp")
                nc.sync.dma_start(
                    sut[:], su[m].rearrange("p (ko c) -> p ko c", c=P))
                for j in range(2):
                    psg = pgp.tile([P, 512], f32, name=f"psgs{m}{j}", tag="pg")
                    for k in range(KH):
                        nc.tensor.matmul(
                            psg[:], sgt[:, k, :],
                            xst[k // 4][:, k % 4, j * 512:(j + 1) * 512],
                            start=(k == 0), stop=(k == KH - 1))
                    st = tmp.tile([P, 512], bf, name=f"sts{m}{j}", tag="tmp")
                    nc.scalar.activation(st[:], psg[:], AF.Silu)
                    psu = pup.tile([P, 512], f32, name=f"psus{m}{j}", tag="pu")
                    for k in range(KH):
                        nc.tensor.matmul(
                            psu[:], sut[:, k, :],
                            xst[k // 4][:, k % 4, j * 512:(j + 1) * 512],
                            start=(k == 0), stop=(k == KH - 1))
                    it = itp.tile([P, 512], bf, name=f"si{m}{j}", tag="it")
                    nc.vector.tensor_mul(it[:], st[:], psu[:])
                    sint[(m, j)] = it
            for M in range(MH):
                sdt = sdp.tile([P, MSI, P], bf, name=f"sdt{M}", tag="sdt")
                with tc.tile_wait_until(0.60):
                    nc.gpsimd.dma_start(
                        sdt[:], sd[M].rearrange("p (ko c) -> p ko c", c=P))
                ot = otp.tile([P, 1024], bf, name=f"ots{M}", tag="ot")
                # last M drains in smaller chunks so the final cast+DMA
                # tail after the last matmul is shorter
                chunks = ([(0, 512), (512, 512)] if M < MH - 1
                          else [(0, 512), (512, 256), (768, 256)])
                for ci, (c0, cw) in enumerate(chunks):
                    j = min(c0 // 512, 1)
                    psy = pyp.tile([P, 512], f32, name=f"psys{M}_{ci}",
                                   tag="py")
                    for K in range(MSI):
                        nc.tensor.matmul(
                            psy[:, :cw], sdt[:, K, :],
                            sint[(K, j)][:, c0 - j * 512:c0 - j * 512 + cw],
                            start=(K == 0), stop=(K == MSI - 1))
                    nc.vector.tensor_copy(ot[:, c0:c0 + cw], psy[:, :cw])
                    nc.scalar.dma_start(
                        ys[M * P:(M + 1) * P, c0:c0 + cw],
                        ot[:, c0:c0 + cw])

    nc.compile()
    return nc


def _get_compiled(T, caps):
    key = (T, tuple(caps))
    if key not in _COMPILED:
        _COMPILED[key] = _build(T, caps)
    return _COMPILED[key]


def _cap_for(maxc):
    maxc = max(int(maxc), 64)
    np_r = max(2, math.ceil(maxc / 2048))
    w = min(512, 2 * math.ceil(maxc / (np_r * 2 * 2)))
    C_cap = np_r * 2 * w
    assert C_cap >= maxc
    return C_cap, w


def kernel(hidden_states, gate_weight, e_score_correction_bias,
           gate_proj, up_proj, down_proj,
           shared_gate_w, shared_up_w, shared_down_w):
    from concourse.bass_utils import run_bass_kernel_spmd

    hs = np.asarray(hidden_states, dtype=np.float32)
    B, S, Hh = hs.shape
    assert Hh == H
    hsf = np.ascontiguousarray(hs.reshape(-1, H))
    T = hsf.shape[0]
    TS = T // 4        # shared-expert token slice (D=2 hybrid shard)
    gate_weight = np.asarray(gate_weight, np.float32)
    bias = np.asarray(e_score_correction_bias, np.float32)
    gate_proj = np.asarray(gate_proj, np.float32)
    up_proj = np.asarray(up_proj, np.float32)
    down_proj = np.asarray(down_proj, np.float32)
    shared_gate_w = np.asarray(shared_gate_w, np.float32)
    shared_up_w = np.asarray(shared_up_w, np.float32)
    shared_down_w = np.asarray(shared_down_w, np.float32)

    # ---- routing on host ----
    topk_idx, topk_w = _gate_host(hsf, gate_weight, bias)
    comb = np.zeros((T, E), np.float32)
    np.add.at(comb, (np.arange(T)[:, None], topk_idx), topk_w)
    sel = np.zeros((T, E), bool)
    sel[np.arange(T)[:, None], topk_idx] = True
    idx_e = [np.nonzero(sel[:, e])[0] for e in range(E)]
    counts = np.array([len(ix) for ix in idx_e])

    # assign experts to (core, slot): slot 0 gets the 8 largest, slot 1 the
    # 8 smallest, so each slot's capacity (uniform across cores under SPMD)
    # hugs its own max count
    order = np.argsort(-counts, kind="stable")
    assign = np.zeros((N_CORES, EXP_PER_CORE), np.int64)
    for c in range(N_CORES):
        assign[c, 0] = order[c]
        assign[c, 1] = order[2 * N_CORES - 1 - c]
    caps = [
        _cap_for(counts[assign[:, 0]].max()),
        _cap_for(counts[assign[:, 1]].max()),
    ]
    slot_base = [0, caps[0][0]]
    C_tot = caps[0][0] + caps[1][0]

    # ---- host-side dispatch (shard + transpose + bf16/fp8 cast) ----
    xsT = np.ascontiguousarray(hsf.T)                       # [H, T] fp32
    xsTb = xsT.astype(BF16)
    xsT8 = np.clip(xsT * SX, -240, 240).astype(FP8)

    MI, MH, MSI, KH = I // P, H // P, SI // (2 * P), H // P
    KC = KH // 2
    SIH = SI // 2      # 1408 shared-intermediate columns per group
    NF8S = [n * P for n in N8S]   # fp8 I-rows per slot's expert

    def tile_gu(wmat, nm):  # [I', H] -> [nm, P, KH*P] : (m, p_h, ko_h*P + c_i)
        return np.ascontiguousarray(
            wmat.reshape(nm, P, KH, P).transpose(0, 3, 2, 1)
        ).reshape(nm, P, KH * P).astype(BF16)

    def tile_gu8(wmat, n8):  # [n8*P, H] fp32 -> [n8, P, KC*2*P] DR layout
        q = np.clip(wmat * SW, -240, 240).astype(FP8)
        # [m, r, c, i, p] -> [m, p, c, i, r]
        a = q.reshape(n8, P, KC, 2, P).transpose(0, 4, 2, 3, 1)
        return np.ascontiguousarray(a).reshape(n8, P, KC * 2 * P)

    def tile_dn(wmat, nk):  # [H, I'] -> [MH, P, nk*P] : (M, p_i, Ko_i*P + c_h)
        return np.ascontiguousarray(
            wmat.reshape(MH, P, nk, P).transpose(0, 3, 2, 1)
        ).reshape(MH, P, nk * P).astype(BF16)

    # shared weights: two column groups (cores 0-3 and 4-7); each core also
    # takes a 1024-token slice, so the shared output is a 2-way partial sum
    sg_g = [tile_gu(shared_gate_w[g * SIH:(g + 1) * SIH], MSI) for g in (0, 1)]
    su_g = [tile_gu(shared_up_w[g * SIH:(g + 1) * SIH], MSI) for g in (0, 1)]
    sd_g = [tile_dn(shared_down_w[:, g * SIH:(g + 1) * SIH], MSI)
            for g in (0, 1)]

    def pack_panels(xmat, NP, cols):
        # [H, NP*cols] -> [NP*4, P, 4*cols], tile np*4+kq holds h rows
        # (kq*4+kk)*128+p and cols [np*cols + c]
        a = xmat.reshape(4, 4, P, NP, cols)          # [kq, kk, p, np, c]
        return np.ascontiguousarray(
            a.transpose(3, 0, 2, 1, 4).reshape(NP * 4, P, 4 * cols)
        )

    def pack_panels8(x8, NP, w, pw):
        # [H, NP*2w] fp8 -> [NP*4, P, 2*2*2pw]: tile np*4+q holds chunks
        # c = 2q+kk (h = c*256 + i*128 + p), cols j*pw + n (n < w)
        a = x8.reshape(8, 2, P, NP, 2, w)            # [c, i, p, np, j, n]
        out = np.zeros((NP, 4, 2, P, 2, 2, pw), FP8)  # [np,q,kk,p,i,j,col]
        out[..., :w] = a.transpose(3, 0, 2, 1, 4, 5).reshape(
            NP, 4, 2, P, 2, 2, w)
        return np.ascontiguousarray(
            out.transpose(0, 1, 3, 2, 4, 5, 6).reshape(NP * 4, P, 2 * 2 * 2 * pw))

    in_maps = []
    for c in range(N_CORES):
        e0, e1 = assign[c]
        xg_pk, xg8_pk = [], []
        for sslot, e in enumerate((e0, e1)):
            C_cap, w = caps[sslot]
            pw = _pw(w)
            NP_R = C_cap // (2 * w)
            xg_c = np.zeros((H, C_cap), BF16)
            xg_c[:, :counts[e]] = xsTb[:, idx_e[e]]
            xg_pk.append(pack_panels(xg_c, NP_R, 2 * w))
            xg8_c = np.zeros((H, C_cap), FP8)
            xg8_c[:, :counts[e]] = xsT8[:, idx_e[e]]
            xg8_pk.append(pack_panels8(xg8_c, NP_R, w, pw))
        wg_c = np.concatenate([tile_gu(gate_proj[e][NF8S[ss]:], MI - N8S[ss])
                               for ss, e in enumerate((e0, e1))])
        wu_c = np.concatenate([tile_gu(up_proj[e][NF8S[ss]:], MI - N8S[ss])
                               for ss, e in enumerate((e0, e1))])
        wg8_c = np.concatenate([tile_gu8(gate_proj[e][:NF8S[ss]], N8S[ss])
                                for ss, e in enumerate((e0, e1))])
        wu8_c = np.concatenate([tile_gu8(up_proj[e][:NF8S[ss]], N8S[ss])
                                for ss, e in enumerate((e0, e1))])
        wd_list = []
        for ss, e in enumerate((e0, e1)):
            dpe = down_proj[e].copy()
            dpe[:, :NF8S[ss]] *= SINV   # descale for fp8 blocks' inter
            wd_list.append(tile_dn(dpe, MI))
        wd_c = np.concatenate(wd_list)
        g, ts = c // 4, c % 4
        in_maps.append({
            "xs": pack_panels(
                np.ascontiguousarray(xsTb[:, ts * TS:(ts + 1) * TS]), 1, TS),
            "xg0": xg_pk[0], "xg1": xg_pk[1],
            "xg80": xg8_pk[0], "xg81": xg8_pk[1],
            "wg": wg_c, "wu": wu_c, "wg8": wg8_c, "wu8": wu8_c, "wd": wd_c,
            "sg": sg_g[g], "su": su_g[g], "sd": sd_g[g],
        })

    nc = _get_compiled(T, caps)

    def _run_and_combine():
        results = run_bass_kernel_spmd(nc, in_maps,
                                       core_ids=list(range(N_CORES)))
        outT = np.zeros((H, T), np.float32)
        for c in range(N_CORES):
            ts = c % 4
            outT[:, ts * TS:(ts + 1) * TS] += \
                results.results[c]["ys"].astype(np.float32)
        for c in range(N_CORES):
            ye = results.results[c]["ye"].astype(np.float32)
            for sslot in range(EXP_PER_CORE):
                e = assign[c, sslot]
                cnt = counts[e]
                if cnt == 0:
                    continue
                b0 = slot_base[sslot]
                we = comb[idx_e[e], e]
                outT[:, idx_e[e]] += ye[:, b0:b0 + cnt] * we[None, :]
        return results, outT

    def _spot_err(outT):
        # exact fp32 recompute of a few tokens; the hybrid device path is
        # within ~3e-2 of this, a corrupted pass is off by orders of
        # magnitude
        errs = []
        for t in (0, T // 3, T // 2, T - 1):
            x = hsf[t]
            acc = np.zeros(H, np.float32)
            for e in topk_idx[t]:
                g = gate_proj[e] @ x
                u = up_proj[e] @ x
                inter = g / (1.0 + np.exp(-g)) * u
                acc += comb[t, e] * (down_proj[e] @ inter)
            sg_ = shared_gate_w @ x
            su_ = shared_up_w @ x
            acc += shared_down_w @ (sg_ / (1.0 + np.exp(-sg_)) * su_)
            errs.append(np.linalg.norm(outT[:, t] - acc)
                        / (np.linalg.norm(acc) + 1e-20))
        return max(errs)

    results, outT = _run_and_combine()
    if _spot_err(outT) > 0.2:   # transient device fault: retry once
        results, outT = _run_and_combine()

    _LAST.clear()
    _LAST.update(nc=nc, in_maps=in_maps, results=results, caps=caps)

    return np.ascontiguousarray(outT.T).reshape(B, S, H).astype(np.float32)
